# revision 2
# baseline (speedup 1.0000x reference)
"""BlockStackingSGN kernel for 8 Trainium2 NeuronCores.

Strategy: data-parallel over batch B=4096 -> 512 rows per core; all MLP
weights replicated. On-chip layout keeps activations transposed
([hidden -> partitions, batch -> free]) so every matmul streams the batch
through the PE with the weight stationary. The three 256->1 output heads
(clear / ontable / AonB) are folded into one PSUM accumulation bank: each
head's weight column is embedded at output-row position r of a [128,128]
stationary operand, so all 80 output rows accumulate into a single
[128, 512] bank and one batched Sigmoid finishes the kernel.
"""

import sys

import numpy as np

sys.path.insert(0, "/opt/trn_rl_repo")

import concourse.bacc as bacc
import concourse.mybir as mybir
import concourse.tile as tile
from concourse.bass_utils import run_bass_kernel_spmd

dt = mybir.dt
AF = mybir.ActivationFunctionType
ALU = mybir.AluOpType

N = 8          # blocks
H = 256        # hidden
B = 4096       # batch
IN = 3 * N     # 24
NCORES = 8
BC = B // NCORES   # 512 batch rows per core
KT = H // 128      # k-tiles per 256-wide contraction
R = N * (N + 2)    # 80 output rows per batch element

F32 = dt.float32
F32R = dt.float32r

_CACHE = {}


def _mm(nc, out, lhsT, rhs, start, stop):
    nc.tensor.matmul(out, lhsT, rhs, start=start, stop=stop)


def _build():
    nc = bacc.Bacc("TRN2", target_bir_lowering=False, debug=False, num_devices=NCORES)

    def din(name, shape, dtype=F32R):
        return nc.dram_tensor(name, list(shape), dtype, kind="ExternalInput")

    d_xT = din("xT", [IN, BC])
    d_oW0 = din("oW0", [N, IN, H])
    d_oW1 = din("oW1", [N, KT, 128, H])
    d_oW2 = din("oW2", [N, KT, 128, H])
    d_ob0 = din("ob0", [N, KT, 128, 1], F32)
    d_ob1 = din("ob1", [N, KT, 128, 1], F32)
    d_ob2 = din("ob2", [N, KT, 128, 1], F32)
    d_cW0 = din("cW0", [KT, 128, H])
    d_cW1 = din("cW1", [KT, 128, H])
    d_cb0 = din("cb0", [KT, 128, 1], F32)
    d_cb1 = din("cb1", [KT, 128, 1], F32)
    d_tW0 = din("tW0", [KT, 128, H])
    d_tW1 = din("tW1", [KT, 128, H])
    d_tb0 = din("tb0", [KT, 128, 1], F32)
    d_tb1 = din("tb1", [KT, 128, 1], F32)
    d_aW0l = din("aW0l", [KT, 128, H])
    d_aW0r = din("aW0r", [KT, 128, H])
    d_aW1 = din("aW1", [KT, 128, H])
    d_ab0 = din("ab0", [KT, 128, 1], F32)
    d_ab1 = din("ab1", [KT, 128, 1], F32)
    # w2 embeddings: [ctype, k] -> [128, 256] with the head column at col 128
    d_w2e = din("w2e", [3, KT, 128, 2 * 128])
    d_finb = din("finb", [128, 1], F32)
    d_out = nc.dram_tensor("outT", [R, BC], F32, kind="ExternalOutput")

    with tile.TileContext(nc) as tc:
        with (
            tc.tile_pool(name="w", bufs=1) as wp,
            tc.tile_pool(name="act", bufs=1) as ap,
            tc.tile_pool(name="wk", bufs=3) as wk,
            tc.tile_pool(name="ps", bufs=4, space="PSUM") as ps,
            tc.tile_pool(name="finp", bufs=1, space="PSUM") as fp,
        ):
            def load(src_ap, shape, tag, dtype=F32R):
                t = wp.tile(list(shape), dtype, tag=tag)
                nc.sync.dma_start(t[:], src_ap)
                return t

            xT = load(d_xT[:], [IN, BC], "xT")
            oW0 = [load(d_oW0[n], [IN, H], f"oW0_{n}") for n in range(N)]
            oW1 = [[load(d_oW1[n, k], [128, H], f"oW1_{n}_{k}") for k in range(KT)] for n in range(N)]
            oW2 = [[load(d_oW2[n, k], [128, H], f"oW2_{n}_{k}") for k in range(KT)] for n in range(N)]
            ob0 = [[load(d_ob0[n, m], [128, 1], f"ob0_{n}_{m}", F32) for m in range(KT)] for n in range(N)]
            ob1 = [[load(d_ob1[n, m], [128, 1], f"ob1_{n}_{m}", F32) for m in range(KT)] for n in range(N)]
            ob2 = [[load(d_ob2[n, m], [128, 1], f"ob2_{n}_{m}", F32) for m in range(KT)] for n in range(N)]
            cW0 = [load(d_cW0[k], [128, H], f"cW0_{k}") for k in range(KT)]
            cW1 = [load(d_cW1[k], [128, H], f"cW1_{k}") for k in range(KT)]
            cb0 = [load(d_cb0[m], [128, 1], f"cb0_{m}", F32) for m in range(KT)]
            cb1 = [load(d_cb1[m], [128, 1], f"cb1_{m}", F32) for m in range(KT)]
            tW0 = [load(d_tW0[k], [128, H], f"tW0_{k}") for k in range(KT)]
            tW1 = [load(d_tW1[k], [128, H], f"tW1_{k}") for k in range(KT)]
            tb0 = [load(d_tb0[m], [128, 1], f"tb0_{m}", F32) for m in range(KT)]
            tb1 = [load(d_tb1[m], [128, 1], f"tb1_{m}", F32) for m in range(KT)]
            aW0l = [load(d_aW0l[k], [128, H], f"aW0l_{k}") for k in range(KT)]
            aW0r = [load(d_aW0r[k], [128, H], f"aW0r_{k}") for k in range(KT)]
            aW1 = [load(d_aW1[k], [128, H], f"aW1_{k}") for k in range(KT)]
            ab0 = [load(d_ab0[m], [128, 1], f"ab0_{m}", F32) for m in range(KT)]
            ab1 = [load(d_ab1[m], [128, 1], f"ab1_{m}", F32) for m in range(KT)]
            w2e = [[load(d_w2e[t_, k], [128, 2 * 128], f"w2e_{t_}_{k}") for k in range(KT)] for t_ in range(3)]
            finb = load(d_finb[:], [128, 1], "finb", F32)

            fin = fp.tile([128, BC], F32, tag="fin")
            n_fin = 2 * KT * N + KT * N * N  # 32 head MMs for clear/table + 128 for AonB
            fin_ct = [0]

            def fin_mm(w2t, r, rhs):
                first = fin_ct[0] == 0
                fin_ct[0] += 1
                last = fin_ct[0] == n_fin
                _mm(nc, fin[:], w2t[:, 128 - r : 256 - r], rhs, start=first, stop=last)

            def layer(w_k, b_m, in_k, func, out_tag):
                """in_k: list of [*, BC] k-tiles; w_k: list of [*, H] weights.
                Returns KT output tiles [128, BC] = func(W.T @ in + b)."""
                outs = []
                for m in range(KT):
                    pst = ps.tile([128, BC], F32, tag="ps")
                    for k in range(len(in_k)):
                        _mm(nc, pst[:], w_k[k][:, m * 128 : (m + 1) * 128], in_k[k][:],
                            start=(k == 0), stop=(k == len(in_k) - 1))
                    ot = wk.tile([128, BC], F32R, tag=out_tag)
                    nc.scalar.activation(ot[:], pst[:], func, bias=b_m[m][:])
                    outs.append(ot)
                return outs

            # ---- object encoders -> encT[n][k] (persistent) ----
            enc = []
            for n in range(N):
                h0 = layer([oW0[n]], ob0[n], [xT], AF.Relu, "h")
                h1 = layer(oW1[n], ob1[n], h0, AF.Relu, "h")
                e = []
                for m in range(KT):
                    pst = ps.tile([128, BC], F32, tag="ps")
                    for k in range(KT):
                        _mm(nc, pst[:], oW2[n][k][:, m * 128 : (m + 1) * 128], h1[k][:],
                            start=(k == 0), stop=(k == KT - 1))
                    et = ap.tile([128, BC], F32R, tag=f"enc_{n}_{m}")
                    nc.scalar.activation(et[:], pst[:], AF.Identity, bias=ob2[n][m][:])
                    e.append(et)
                enc.append(e)

            # ---- clear / ontable predicates -> head rows i*10+8 / i*10+9 ----
            for n in range(N):
                for (W0, W1_, b0_, b1_, w2idx, r) in (
                    (cW0, cW1, cb0, cb1, 0, n * 10 + 8),
                    (tW0, tW1, tb0, tb1, 1, n * 10 + 9),
                ):
                    y0 = layer(W0, b0_, enc[n], AF.Relu, "h")
                    y1 = layer(W1_, b1_, y0, AF.Relu, "h")
                    for k in range(KT):
                        fin_mm(w2e[w2idx][k], r, y1[k][:])

            # ---- AonB pair-input halves (bias a_b0 folded into left) ----
            al, ar = [], []
            for n in range(N):
                a = []
                for m in range(KT):
                    pst = ps.tile([128, BC], F32, tag="ps")
                    for k in range(KT):
                        _mm(nc, pst[:], aW0l[k][:, m * 128 : (m + 1) * 128], enc[n][k][:],
                            start=(k == 0), stop=(k == KT - 1))
                    t = ap.tile([128, BC], F32R, tag=f"al_{n}_{m}")
                    nc.scalar.activation(t[:], pst[:], AF.Identity, bias=ab0[m][:])
                    a.append(t)
                al.append(a)
                a = []
                for m in range(KT):
                    pst = ps.tile([128, BC], F32, tag="ps")
                    for k in range(KT):
                        _mm(nc, pst[:], aW0r[k][:, m * 128 : (m + 1) * 128], enc[n][k][:],
                            start=(k == 0), stop=(k == KT - 1))
                    t = ap.tile([128, BC], F32R, tag=f"ar_{n}_{m}")
                    nc.scalar.activation(t[:], pst[:], AF.Copy)
                    a.append(t)
                ar.append(a)

            # ---- all (i, j) pairs ----
            for i in range(N):
                for j in range(N):
                    r = i * 10 + j
                    ph = []
                    for k in range(KT):
                        pht = wk.tile([128, BC], F32R, tag=f"ph{k}")
                        nc.vector.tensor_tensor(pht[:], al[i][k][:], ar[j][k][:], ALU.add)
                        if k == 0:
                            nc.vector.tensor_scalar(pht[:], pht[:], 0.0, None, ALU.max)
                        else:
                            nc.scalar.activation(pht[:], pht[:], AF.Relu)
                        ph.append(pht)
                    y = []
                    for m in range(KT):
                        pst = ps.tile([128, BC], F32, tag="ps")
                        for k in range(KT):
                            _mm(nc, pst[:], aW1[k][:, m * 128 : (m + 1) * 128], ph[k][:],
                                start=(k == 0), stop=(k == KT - 1))
                        yt = wk.tile([128, BC], F32R, tag=f"y{m}")
                        nc.scalar.activation(yt[:], pst[:], AF.Relu, bias=ab1[m][:])
                        y.append(yt)
                    for k in range(KT):
                        fin_mm(w2e[2][k], r, y[k][:])

            assert fin_ct[0] == n_fin

            # ---- batched sigmoid over all 80 head rows + store ----
            outT = wk.tile([128, BC], F32, tag="outT")
            nc.scalar.activation(outT[:], fin[:], AF.Sigmoid, bias=finb[:])
            nc.sync.dma_start(d_out[:], outT[:R, :])

    nc.compile()
    return nc


def _prep_inputs(inputs):
    f = lambda a: np.ascontiguousarray(np.asarray(a), dtype=np.float32)
    x = f(inputs["x"])
    common = {
        "oW0": f(inputs["o_W0"]),
        "oW1": f(inputs["o_W1"]).reshape(N, KT, 128, H),
        "oW2": f(inputs["o_W2"]).reshape(N, KT, 128, H),
        "ob0": f(inputs["o_b0"]).reshape(N, KT, 128, 1),
        "ob1": f(inputs["o_b1"]).reshape(N, KT, 128, 1),
        "ob2": f(inputs["o_b2"]).reshape(N, KT, 128, 1),
        "cW0": f(inputs["c_W0"]).reshape(KT, 128, H),
        "cW1": f(inputs["c_W1"]).reshape(KT, 128, H),
        "cb0": f(inputs["c_b0"]).reshape(KT, 128, 1),
        "cb1": f(inputs["c_b1"]).reshape(KT, 128, 1),
        "tW0": f(inputs["t_W0"]).reshape(KT, 128, H),
        "tW1": f(inputs["t_W1"]).reshape(KT, 128, H),
        "tb0": f(inputs["t_b0"]).reshape(KT, 128, 1),
        "tb1": f(inputs["t_b1"]).reshape(KT, 128, 1),
        "aW0l": f(inputs["a_W0"][:H]).reshape(KT, 128, H),
        "aW0r": f(inputs["a_W0"][H:]).reshape(KT, 128, H),
        "aW1": f(inputs["a_W1"]).reshape(KT, 128, H),
        "ab0": f(inputs["a_b0"]).reshape(KT, 128, 1),
        "ab1": f(inputs["a_b1"]).reshape(KT, 128, 1),
    }
    w2e = np.zeros((3, KT, 128, 2 * 128), np.float32)
    for t_, w2 in enumerate((inputs["c_W2"], inputs["t_W2"], inputs["a_W2"])):
        w2 = f(w2)[:, 0]
        for k in range(KT):
            w2e[t_, k, :, 128] = w2[k * 128 : (k + 1) * 128]
    common["w2e"] = w2e
    finb = np.zeros((128, 1), np.float32)
    for i in range(N):
        for j in range(N):
            finb[i * 10 + j, 0] = f(inputs["a_b2"])[0]
        finb[i * 10 + 8, 0] = f(inputs["c_b2"])[0]
        finb[i * 10 + 9, 0] = f(inputs["t_b2"])[0]
    common["finb"] = finb

    xT = np.ascontiguousarray(x.T)  # (24, 4096)
    in_maps = []
    for c in range(NCORES):
        m = dict(common)
        m["xT"] = np.ascontiguousarray(xT[:, c * BC : (c + 1) * BC])
        in_maps.append(m)
    return in_maps


def run(inputs, trace=False, **kw):
    if "nc" not in _CACHE:
        _CACHE["nc"] = _build()
    nc = _CACHE["nc"]
    in_maps = _prep_inputs(inputs)
    res = run_bass_kernel_spmd(nc, in_maps, list(range(NCORES)), trace=trace, **kw)
    out = np.concatenate([res.results[c]["outT"].T for c in range(NCORES)], axis=0)
    return out.astype(np.float32), res


def kernel(**inputs) -> np.ndarray:
    out, _ = run(inputs, trace=False)
    return out


# revision 3
# speedup vs baseline: 1.0265x; 1.0265x over previous
"""BlockStackingSGN kernel for 8 Trainium2 NeuronCores.

Strategy: data-parallel over batch B=4096 -> 512 rows per core; all MLP
weights replicated. On-chip layout keeps activations transposed
([hidden -> partitions, batch -> free]) so every matmul streams the batch
through the PE with the weight stationary (bf16 operands, fp32 PSUM
accumulation). The three 256->1 output heads (clear / ontable / AonB) are
folded into one PSUM accumulation bank: each head's weight column is
embedded at output-row position r of a [128,128] stationary operand, so
all 80 output rows accumulate into a single [128, 512] bank and one
batched Sigmoid finishes the kernel. Elementwise work is spread across
the Scalar, Vector, and GpSimd engines to keep them all under the PE's
span.
"""

import sys

import numpy as np

sys.path.insert(0, "/opt/trn_rl_repo")

import concourse.bacc as bacc
import concourse.mybir as mybir
import concourse.tile as tile
from concourse.bass_utils import run_bass_kernel_spmd

dt = mybir.dt
AF = mybir.ActivationFunctionType
ALU = mybir.AluOpType

N = 8          # blocks
H = 256        # hidden
B = 4096       # batch
IN = 3 * N     # 24
NCORES = 8
BC = B // NCORES   # 512 batch rows per core
KT = H // 128      # k-tiles per 256-wide contraction
R = N * (N + 2)    # 80 output rows per batch element

F32 = dt.float32
BF16 = dt.bfloat16
W = BC  # free width of one k/m tile

_CACHE = {}


def _build():
    nc = bacc.Bacc("TRN2", target_bir_lowering=False, debug=False, num_devices=NCORES)

    def din(name, shape, dtype=BF16):
        return nc.dram_tensor(name, list(shape), dtype, kind="ExternalInput")

    d_xT = din("xT", [IN, BC])
    d_oW0 = din("oW0", [N, IN, H])
    d_oW1 = din("oW1", [N, KT, 128, H])
    d_oW2 = din("oW2", [N, KT, 128, H])
    d_ob0 = din("ob0", [N, KT, 128, 1], F32)
    d_ob1 = din("ob1", [N, KT, 128, 1], F32)
    d_ob2 = din("ob2", [N, KT, 128, 1], F32)
    d_cW0 = din("cW0", [KT, 128, H])
    d_cW1 = din("cW1", [KT, 128, H])
    d_cb0 = din("cb0", [KT, 128, 1], F32)
    d_cb1 = din("cb1", [KT, 128, 1], F32)
    d_tW0 = din("tW0", [KT, 128, H])
    d_tW1 = din("tW1", [KT, 128, H])
    d_tb0 = din("tb0", [KT, 128, 1], F32)
    d_tb1 = din("tb1", [KT, 128, 1], F32)
    d_aW0l = din("aW0l", [KT, 128, H])
    d_aW0r = din("aW0r", [KT, 128, H])
    d_aW1 = din("aW1", [KT, 128, H])
    d_ab0 = din("ab0", [KT, 128, 1], F32)
    d_ab1 = din("ab1", [KT, 128, 1], F32)
    # w2 embeddings: [ctype, k, parity] -> [128, 256], head column at 128+parity
    d_w2e = din("w2e", [3, KT, 2, 128, 2 * 128])
    d_finb = din("finb", [128, 1], F32)
    d_out = nc.dram_tensor("outT", [R, BC], F32, kind="ExternalOutput")

    with tile.TileContext(nc) as tc:
        with (
            tc.tile_pool(name="w", bufs=1) as wp,
            tc.tile_pool(name="act", bufs=1) as ap,
            tc.tile_pool(name="wk", bufs=3) as wk,
            tc.tile_pool(name="ps", bufs=4, space="PSUM") as ps,
            tc.tile_pool(name="finp", bufs=1, space="PSUM") as fp,
        ):
            def load(src_ap, shape, tag, dtype=BF16):
                t = wp.tile(list(shape), dtype, tag=tag)
                nc.sync.dma_start(t[:], src_ap)
                return t

            xT = load(d_xT[:], [IN, BC], "xT")
            oW0 = [load(d_oW0[n], [IN, H], f"oW0_{n}") for n in range(N)]
            oW1 = [[load(d_oW1[n, k], [128, H], f"oW1_{n}_{k}") for k in range(KT)] for n in range(N)]
            oW2 = [[load(d_oW2[n, k], [128, H], f"oW2_{n}_{k}") for k in range(KT)] for n in range(N)]
            ob0 = [[load(d_ob0[n, m], [128, 1], f"ob0_{n}_{m}", F32) for m in range(KT)] for n in range(N)]
            ob1 = [[load(d_ob1[n, m], [128, 1], f"ob1_{n}_{m}", F32) for m in range(KT)] for n in range(N)]
            ob2 = [[load(d_ob2[n, m], [128, 1], f"ob2_{n}_{m}", F32) for m in range(KT)] for n in range(N)]
            cW0 = [load(d_cW0[k], [128, H], f"cW0_{k}") for k in range(KT)]
            cW1 = [load(d_cW1[k], [128, H], f"cW1_{k}") for k in range(KT)]
            cb0 = [load(d_cb0[m], [128, 1], f"cb0_{m}", F32) for m in range(KT)]
            cb1 = [load(d_cb1[m], [128, 1], f"cb1_{m}", F32) for m in range(KT)]
            tW0 = [load(d_tW0[k], [128, H], f"tW0_{k}") for k in range(KT)]
            tW1 = [load(d_tW1[k], [128, H], f"tW1_{k}") for k in range(KT)]
            tb0 = [load(d_tb0[m], [128, 1], f"tb0_{m}", F32) for m in range(KT)]
            tb1 = [load(d_tb1[m], [128, 1], f"tb1_{m}", F32) for m in range(KT)]
            aW0l = [load(d_aW0l[k], [128, H], f"aW0l_{k}") for k in range(KT)]
            aW0r = [load(d_aW0r[k], [128, H], f"aW0r_{k}") for k in range(KT)]
            aW1 = [load(d_aW1[k], [128, H], f"aW1_{k}") for k in range(KT)]
            ab0 = [load(d_ab0[m], [128, 1], f"ab0_{m}", F32) for m in range(KT)]
            ab1 = [load(d_ab1[m], [128, 1], f"ab1_{m}", F32) for m in range(KT)]
            w2e = [[[load(d_w2e[t_, k, p], [128, 2 * 128], f"w2e_{t_}_{k}_{p}")
                     for p in range(2)] for k in range(KT)] for t_ in range(3)]
            finb = load(d_finb[:], [128, 1], "finb", F32)

            fin = fp.tile([128, BC], F32, tag="fin")
            n_fin = 2 * KT * N + KT * N * N
            fin_ct = [0]

            def fin_mm(w2t_parities, r, rhs):
                first = fin_ct[0] == 0
                fin_ct[0] += 1
                last = fin_ct[0] == n_fin
                p = r % 2
                w2t = w2t_parities[p]
                nc.tensor.matmul(fin[:], w2t[:, 128 + p - r : 256 + p - r], rhs,
                                 start=first, stop=last)

            # round-robin dispatch of PSUM->SBUF evacuations over ACT / DVE
            evac_ct = [0]

            def evac(out_ap, psum_ap, bias, relu):
                evac_ct[0] += 1
                if evac_ct[0] % 2 == 0:
                    func = AF.Relu if relu else (AF.Identity if bias is not None else AF.Copy)
                    if bias is not None:
                        nc.scalar.activation(out_ap, psum_ap, func, bias=bias[:])
                    else:
                        nc.scalar.activation(out_ap, psum_ap, func)
                else:
                    if relu:
                        b = bias[:] if bias is not None else 0.0
                        nc.vector.tensor_scalar(out_ap, psum_ap, b, 0.0, ALU.add, ALU.max)
                    elif bias is not None:
                        nc.vector.tensor_scalar(out_ap, psum_ap, bias[:], None, ALU.add)
                    else:
                        nc.vector.tensor_copy(out_ap, psum_ap)

            def layer(w_k, b_m, in_wide, in_k_parts, relu, out_pool, out_tag):
                """One 256-wide (or IN-wide) layer into a [128, KT*W] tile.
                in_wide: input tile [128, KT*W] (or None, with in_k_parts a
                list of (tile, slice) k-parts)."""
                out = out_pool.tile([128, KT * W], BF16, tag=out_tag)
                if in_wide is not None:
                    in_k_parts = [(in_wide, k) for k in range(KT)]
                for m in range(KT):
                    pst = ps.tile([128, BC], F32, tag="ps")
                    nk = len(in_k_parts)
                    for ki, (t, k) in enumerate(in_k_parts):
                        rhs = t[:, k * W : (k + 1) * W] if k is not None else t[:]
                        nc.tensor.matmul(pst[:], w_k[ki][:, m * 128 : (m + 1) * 128], rhs,
                                         start=(ki == 0), stop=(ki == nk - 1))
                    evac(out[:, m * W : (m + 1) * W], pst[:], b_m[m] if b_m else None, relu)
                return out

            # ---- object encoders -> enc[n] [128, 2W] (persistent) ----
            enc = []
            for n in range(N):
                h0 = layer([oW0[n]], ob0[n], None, [(xT, None)], True, wk, "h")
                h1 = layer(oW1[n], ob1[n], h0, None, True, wk, "h")
                e = layer(oW2[n], ob2[n], h1, None, False, ap, f"enc_{n}")
                enc.append(e)

            # ---- clear / ontable predicates -> head rows i*10+8 / i*10+9 ----
            for n in range(N):
                for (W0, W1_, b0_, b1_, w2idx, r) in (
                    (cW0, cW1, cb0, cb1, 0, n * 10 + 8),
                    (tW0, tW1, tb0, tb1, 1, n * 10 + 9),
                ):
                    y0 = layer(W0, b0_, enc[n], None, True, wk, "h")
                    y1 = layer(W1_, b1_, y0, None, True, wk, "h")
                    for k in range(KT):
                        fin_mm(w2e[w2idx][k], r, y1[:, k * W : (k + 1) * W])

            # ---- AonB pair-input halves (bias a_b0 folded into left) ----
            al = [layer(aW0l, ab0, enc[n], None, False, ap, f"al_{n}") for n in range(N)]
            ar = [layer(aW0r, None, enc[n], None, False, ap, f"ar_{n}") for n in range(N)]

            # ---- all (i, j) pairs ----
            add_ct = [0]
            for i in range(N):
                for j in range(N):
                    r = i * 10 + j
                    ph = wk.tile([128, KT * W], BF16, tag="ph")
                    add_ct[0] += 1
                    if add_ct[0] % 2 == 0:
                        nc.gpsimd.tensor_tensor(ph[:], al[i][:], ar[j][:], ALU.add)
                    else:
                        nc.vector.tensor_tensor(ph[:], al[i][:], ar[j][:], ALU.add)
                    nc.vector.tensor_scalar(ph[:], ph[:], 0.0, None, ALU.max)
                    y = layer(aW1, ab1, ph, None, True, wk, "y")
                    for k in range(KT):
                        fin_mm(w2e[2][k], r, y[:, k * W : (k + 1) * W])

            assert fin_ct[0] == n_fin

            # ---- batched sigmoid over all 80 head rows + store ----
            outT = wk.tile([128, BC], F32, tag="outT")
            nc.scalar.activation(outT[:], fin[:], AF.Sigmoid, bias=finb[:])
            nc.sync.dma_start(d_out[:], outT[:R, :])

    nc.compile()
    return nc


def _prep_inputs(inputs):
    import ml_dtypes

    bf = ml_dtypes.bfloat16
    f = lambda a: np.ascontiguousarray(np.asarray(a), dtype=np.float32)
    fb = lambda a: np.ascontiguousarray(np.asarray(a, dtype=np.float32).astype(bf))
    x = f(inputs["x"])
    common = {
        "oW0": fb(inputs["o_W0"]),
        "oW1": fb(inputs["o_W1"]).reshape(N, KT, 128, H),
        "oW2": fb(inputs["o_W2"]).reshape(N, KT, 128, H),
        "ob0": f(inputs["o_b0"]).reshape(N, KT, 128, 1),
        "ob1": f(inputs["o_b1"]).reshape(N, KT, 128, 1),
        "ob2": f(inputs["o_b2"]).reshape(N, KT, 128, 1),
        "cW0": fb(inputs["c_W0"]).reshape(KT, 128, H),
        "cW1": fb(inputs["c_W1"]).reshape(KT, 128, H),
        "cb0": f(inputs["c_b0"]).reshape(KT, 128, 1),
        "cb1": f(inputs["c_b1"]).reshape(KT, 128, 1),
        "tW0": fb(inputs["t_W0"]).reshape(KT, 128, H),
        "tW1": fb(inputs["t_W1"]).reshape(KT, 128, H),
        "tb0": f(inputs["t_b0"]).reshape(KT, 128, 1),
        "tb1": f(inputs["t_b1"]).reshape(KT, 128, 1),
        "aW0l": fb(inputs["a_W0"][:H]).reshape(KT, 128, H),
        "aW0r": fb(inputs["a_W0"][H:]).reshape(KT, 128, H),
        "aW1": fb(inputs["a_W1"]).reshape(KT, 128, H),
        "ab0": f(inputs["a_b0"]).reshape(KT, 128, 1),
        "ab1": f(inputs["a_b1"]).reshape(KT, 128, 1),
    }
    w2e = np.zeros((3, KT, 2, 128, 2 * 128), bf)
    for t_, w2 in enumerate((inputs["c_W2"], inputs["t_W2"], inputs["a_W2"])):
        w2 = np.asarray(w2, np.float32).astype(bf)[:, 0]
        for k in range(KT):
            for p in range(2):
                w2e[t_, k, p, :, 128 + p] = w2[k * 128 : (k + 1) * 128]
    common["w2e"] = w2e
    finb = np.zeros((128, 1), np.float32)
    for i in range(N):
        for j in range(N):
            finb[i * 10 + j, 0] = np.float32(np.asarray(inputs["a_b2"])[0])
        finb[i * 10 + 8, 0] = np.float32(np.asarray(inputs["c_b2"])[0])
        finb[i * 10 + 9, 0] = np.float32(np.asarray(inputs["t_b2"])[0])
    common["finb"] = finb

    xT = np.ascontiguousarray(x.T)  # (24, 4096)
    in_maps = []
    for c in range(NCORES):
        m = dict(common)
        m["xT"] = np.ascontiguousarray(xT[:, c * BC : (c + 1) * BC]).astype(bf)
        in_maps.append(m)
    return in_maps


def run(inputs, trace=False, **kw):
    if "nc" not in _CACHE:
        _CACHE["nc"] = _build()
    nc = _CACHE["nc"]
    in_maps = _prep_inputs(inputs)
    res = run_bass_kernel_spmd(nc, in_maps, list(range(NCORES)), trace=trace, **kw)
    out = np.concatenate([res.results[c]["outT"].T for c in range(NCORES)], axis=0)
    return out.astype(np.float32), res


def kernel(**inputs) -> np.ndarray:
    out, _ = run(inputs, trace=False)
    return out


# revision 4
# speedup vs baseline: 1.1479x; 1.1183x over previous
"""BlockStackingSGN kernel for 8 Trainium2 NeuronCores.

Strategy: data-parallel over batch B=4096 -> 512 rows per core; all MLP
weights replicated. On-chip layout keeps activations transposed
([hidden -> partitions, batch -> free]) so every matmul streams the batch
through the PE with the weight stationary (bf16 operands, fp32 PSUM
accumulation). The three 256->1 output heads (clear / ontable / AonB) are
folded into one PSUM accumulation bank: each head's weight column is
embedded at output-row position r of a [128,128] stationary operand, so
all 80 output rows accumulate into a single [128, 512] bank and one
batched Sigmoid finishes the kernel. Elementwise work is spread across
the Scalar, Vector, and GpSimd engines to keep them all under the PE's
span; all 128-partition weights travel in one packed SBUF tile DMA'd in
parallel chunks.
"""

import sys

import numpy as np

sys.path.insert(0, "/opt/trn_rl_repo")

import concourse.bacc as bacc
import concourse.mybir as mybir
import concourse.tile as tile
from concourse.bass_utils import run_bass_kernel_spmd

dt = mybir.dt
AF = mybir.ActivationFunctionType
ALU = mybir.AluOpType

N = 8          # blocks
H = 256        # hidden
B = 4096       # batch
IN = 3 * N     # 24
NCORES = 8
BC = B // NCORES   # 512 batch rows per core
KT = H // 128      # k-tiles per 256-wide contraction
R = N * (N + 2)    # 80 output rows per batch element

F32 = dt.float32
BF16 = dt.bfloat16
W = BC

_CACHE = {}


def _wb_layout():
    """Column layout of the packed [128, ncols] bf16 weight tile.
    Order doubles as DMA arrival order: block-0 weights first."""
    keys = []
    for n in range(N):
        for k in range(KT):
            keys.append(("oW1", n, k))
            keys.append(("oW2", n, k))
    for nm in ("cW0", "cW1", "tW0", "tW1"):
        for k in range(KT):
            keys.append((nm, k))
    for nm in ("aW0l", "aW0r", "aW1"):
        for k in range(KT):
            keys.append((nm, k))
    for t_ in range(3):
        for k in range(KT):
            for p in range(2):
                keys.append(("w2e", t_, k, p))
    off = {}
    col = 0
    for key in keys:
        off[key] = col
        col += H  # every packed tile is 256 columns wide
    return off, col


WB_OFF, WB_COLS = _wb_layout()

# bias tile column layout ([128, nb] fp32)
def _bias_layout():
    keys = []
    for n in range(N):
        for nm in ("ob0", "ob1", "ob2"):
            for m in range(KT):
                keys.append((nm, n, m))
    for nm in ("cb0", "cb1", "tb0", "tb1", "ab0", "ab1"):
        for m in range(KT):
            keys.append((nm, m))
    keys.append(("finb",))
    return {k: i for i, k in enumerate(keys)}, len(keys)


BIAS_OFF, BIAS_COLS = _bias_layout()

N_DMA_CHUNKS = 8


def _build():
    nc = bacc.Bacc("TRN2", target_bir_lowering=False, debug=False, num_devices=NCORES)

    d_xT = nc.dram_tensor("xT", [IN, BC], BF16, kind="ExternalInput")
    d_wb = nc.dram_tensor("wb", [128, WB_COLS], BF16, kind="ExternalInput")
    d_ow0 = nc.dram_tensor("ow0", [IN, N * H], BF16, kind="ExternalInput")
    d_bias = nc.dram_tensor("bias", [128, BIAS_COLS], F32, kind="ExternalInput")
    d_out = nc.dram_tensor("outT", [R, BC], F32, kind="ExternalOutput")

    with tile.TileContext(nc) as tc:
        with (
            tc.tile_pool(name="w", bufs=1) as wp,
            tc.tile_pool(name="act", bufs=1) as ap,
            tc.tile_pool(name="wk", bufs=3) as wk,
            tc.tile_pool(name="ps", bufs=4, space="PSUM") as ps,
            tc.tile_pool(name="finp", bufs=1, space="PSUM") as fp,
        ):
            xT = wp.tile([IN, BC], BF16, tag="xT")
            nc.sync.dma_start(xT[:], d_xT[:])
            bias = wp.tile([128, BIAS_COLS], F32, tag="bias")
            nc.sync.dma_start(bias[:], d_bias[:])
            ow0 = wp.tile([IN, N * H], BF16, tag="ow0")
            nc.gpsimd.dma_start(ow0[:], d_ow0[:])

            wb = wp.tile([128, WB_COLS], BF16, tag="wb")
            chunk = WB_COLS // N_DMA_CHUNKS
            assert chunk * N_DMA_CHUNKS == WB_COLS
            for c in range(N_DMA_CHUNKS):
                eng = nc.sync if c % 2 == 0 else nc.gpsimd
                sl = slice(c * chunk, (c + 1) * chunk)
                eng.dma_start(wb[:, sl], d_wb[:, sl])

            def wslice(*key):
                o = WB_OFF[key]
                return lambda m: wb[:, o + m * 128 : o + (m + 1) * 128]

            def bcol(*key):
                i = BIAS_OFF[key]
                return bias[:, i : i + 1]

            fin = fp.tile([128, BC], F32, tag="fin")
            n_fin = 2 * KT * N + KT * N * N
            fin_ct = [0]

            def fin_mm(t_, k, r, rhs):
                first = fin_ct[0] == 0
                fin_ct[0] += 1
                last = fin_ct[0] == n_fin
                p = r % 2
                o = WB_OFF[("w2e", t_, k, p)]
                lhsT = wb[:, o + 128 + p - r : o + 256 + p - r]
                nc.tensor.matmul(fin[:], lhsT, rhs, start=first, stop=last)

            # weighted round-robin of PSUM evacuations: 3 ACT : 2 DVE
            evac_ct = [0]
            EV_PAT = "ADADA"

            def evac(out_ap, psum_ap, bias_ap, relu):
                e = EV_PAT[evac_ct[0] % len(EV_PAT)]
                evac_ct[0] += 1
                if e == "A":
                    func = AF.Relu if relu else (AF.Identity if bias_ap is not None else AF.Copy)
                    if bias_ap is not None:
                        nc.scalar.activation(out_ap, psum_ap, func, bias=bias_ap)
                    else:
                        nc.scalar.activation(out_ap, psum_ap, func)
                else:
                    if relu:
                        b = bias_ap if bias_ap is not None else 0.0
                        nc.vector.tensor_scalar(out_ap, psum_ap, b, 0.0, ALU.add, ALU.max)
                    elif bias_ap is not None:
                        nc.vector.tensor_scalar(out_ap, psum_ap, bias_ap, None, ALU.add)
                    else:
                        nc.vector.tensor_copy(out_ap, psum_ap)

            def layer(wgt, bias_m, in_tile, in_parts, relu, out_pool, out_tag):
                """One 256-out layer -> [128, KT*W] tile.
                wgt(k) -> fn m -> lhsT AP; in_parts: list of rhs APs (k-tiles)."""
                out = out_pool.tile([128, KT * W], BF16, tag=out_tag)
                if in_parts is None:
                    in_parts = [in_tile[:, k * W : (k + 1) * W] for k in range(KT)]
                for m in range(KT):
                    pst = ps.tile([128, BC], F32, tag="ps")
                    for ki, rhs in enumerate(in_parts):
                        nc.tensor.matmul(pst[:], wgt(ki)(m), rhs,
                                         start=(ki == 0), stop=(ki == len(in_parts) - 1))
                    evac(out[:, m * W : (m + 1) * W], pst[:],
                         bias_m(m) if bias_m else None, relu)
                return out

            # ---- object encoders -> enc[n] [128, 2W] (persistent) ----
            enc = []
            for n in range(N):
                ow0_l = lambda n=n: (lambda ki: (lambda m: ow0[:, n * H + m * 128 : n * H + (m + 1) * 128]))
                h0 = layer(ow0_l(), lambda m, n=n: bcol("ob0", n, m), None, [xT[:]], True, wk, "h")
                h1 = layer(lambda ki, n=n: wslice("oW1", n, ki), lambda m, n=n: bcol("ob1", n, m),
                           h0, None, True, wk, "h")
                e = layer(lambda ki, n=n: wslice("oW2", n, ki), lambda m, n=n: bcol("ob2", n, m),
                          h1, None, False, ap, f"enc_{n}")
                enc.append(e)

            # ---- clear / ontable predicates -> head rows i*10+8 / i*10+9 ----
            for n in range(N):
                for (w0nm, w1nm, b0nm, b1nm, w2idx, r) in (
                    ("cW0", "cW1", "cb0", "cb1", 0, n * 10 + 8),
                    ("tW0", "tW1", "tb0", "tb1", 1, n * 10 + 9),
                ):
                    y0 = layer(lambda ki, w0nm=w0nm: wslice(w0nm, ki),
                               lambda m, b0nm=b0nm: bcol(b0nm, m), enc[n], None, True, wk, "h")
                    y1 = layer(lambda ki, w1nm=w1nm: wslice(w1nm, ki),
                               lambda m, b1nm=b1nm: bcol(b1nm, m), y0, None, True, wk, "h")
                    for k in range(KT):
                        fin_mm(w2idx, k, r, y1[:, k * W : (k + 1) * W])

            # ---- AonB pair-input halves (bias a_b0 folded into left) ----
            al = [layer(lambda ki: wslice("aW0l", ki), lambda m: bcol("ab0", m),
                        enc[n], None, False, ap, f"al_{n}") for n in range(N)]
            ar = [layer(lambda ki: wslice("aW0r", ki), None,
                        enc[n], None, False, ap, f"ar_{n}") for n in range(N)]

            # ---- all (i, j) pairs ----
            for i in range(N):
                for j in range(N):
                    r = i * 10 + j
                    phs = wk.tile([128, KT * W], BF16, tag="phs")
                    nc.gpsimd.tensor_tensor(phs[:], al[i][:], ar[j][:], ALU.add)
                    ph = wk.tile([128, KT * W], BF16, tag="ph")
                    nc.vector.tensor_scalar(ph[:], phs[:], 0.0, None, ALU.max)
                    y = layer(lambda ki: wslice("aW1", ki), lambda m: bcol("ab1", m),
                              ph, None, True, wk, "y")
                    for k in range(KT):
                        fin_mm(2, k, r, y[:, k * W : (k + 1) * W])

            assert fin_ct[0] == n_fin

            # ---- batched sigmoid over all 80 head rows + store ----
            outT = wk.tile([128, BC], F32, tag="outT")
            nc.scalar.activation(outT[:], fin[:], AF.Sigmoid, bias=bcol("finb"))
            nc.sync.dma_start(d_out[:], outT[:R, :])

    nc.compile()
    return nc


def _prep_inputs(inputs):
    import ml_dtypes

    bf = ml_dtypes.bfloat16
    f32a = lambda a: np.asarray(a, dtype=np.float32)

    wbv = np.zeros((128, WB_COLS), bf)

    def put(key, arr):  # arr: [128, 256] fp32
        o = WB_OFF[key]
        wbv[:, o : o + H] = arr.astype(bf)

    oW1 = f32a(inputs["o_W1"])
    oW2 = f32a(inputs["o_W2"])
    for n in range(N):
        for k in range(KT):
            put(("oW1", n, k), oW1[n, k * 128 : (k + 1) * 128])
            put(("oW2", n, k), oW2[n, k * 128 : (k + 1) * 128])
    for nm, src in (("cW0", "c_W0"), ("cW1", "c_W1"), ("tW0", "t_W0"), ("tW1", "t_W1")):
        a = f32a(inputs[src])
        for k in range(KT):
            put((nm, k), a[k * 128 : (k + 1) * 128])
    aW0 = f32a(inputs["a_W0"])
    for k in range(KT):
        put(("aW0l", k), aW0[k * 128 : (k + 1) * 128])
        put(("aW0r", k), aW0[H + k * 128 : H + (k + 1) * 128])
    aW1 = f32a(inputs["a_W1"])
    for k in range(KT):
        put(("aW1", k), aW1[k * 128 : (k + 1) * 128])
    for t_, src in enumerate(("c_W2", "t_W2", "a_W2")):
        w2 = f32a(inputs[src])[:, 0].astype(bf)
        for k in range(KT):
            for p in range(2):
                o = WB_OFF[("w2e", t_, k, p)]
                wbv[:, o + 128 + p] = w2[k * 128 : (k + 1) * 128]

    biasv = np.zeros((128, BIAS_COLS), np.float32)

    def putb(key, vec128):
        biasv[:, BIAS_OFF[key]] = vec128

    for n in range(N):
        for nm, src in (("ob0", "o_b0"), ("ob1", "o_b1"), ("ob2", "o_b2")):
            a = f32a(inputs[src])[n]
            for m in range(KT):
                putb((nm, n, m), a[m * 128 : (m + 1) * 128])
    for nm, src in (("cb0", "c_b0"), ("cb1", "c_b1"), ("tb0", "t_b0"),
                    ("tb1", "t_b1"), ("ab0", "a_b0"), ("ab1", "a_b1")):
        a = f32a(inputs[src])
        for m in range(KT):
            putb((nm, m), a[m * 128 : (m + 1) * 128])
    finb = np.zeros(128, np.float32)
    for i in range(N):
        finb[i * 10 : i * 10 + 8] = f32a(inputs["a_b2"])[0]
        finb[i * 10 + 8] = f32a(inputs["c_b2"])[0]
        finb[i * 10 + 9] = f32a(inputs["t_b2"])[0]
    putb(("finb",), finb)

    ow0v = np.zeros((IN, N * H), bf)
    oW0 = f32a(inputs["o_W0"])
    for n in range(N):
        ow0v[:, n * H : (n + 1) * H] = oW0[n].astype(bf)

    xT = np.ascontiguousarray(f32a(inputs["x"]).T)  # (24, 4096)
    common = {"wb": wbv, "ow0": ow0v, "bias": biasv}
    in_maps = []
    for c in range(NCORES):
        m = dict(common)
        m["xT"] = np.ascontiguousarray(xT[:, c * BC : (c + 1) * BC]).astype(bf)
        in_maps.append(m)
    return in_maps


def run(inputs, trace=False, **kw):
    if "nc" not in _CACHE:
        _CACHE["nc"] = _build()
    nc = _CACHE["nc"]
    in_maps = _prep_inputs(inputs)
    res = run_bass_kernel_spmd(nc, in_maps, list(range(NCORES)), trace=trace, **kw)
    out = np.concatenate([res.results[c]["outT"].T for c in range(NCORES)], axis=0)
    return out.astype(np.float32), res


def kernel(**inputs) -> np.ndarray:
    out, _ = run(inputs, trace=False)
    return out


# revision 6
# speedup vs baseline: 1.3500x; 1.1761x over previous
"""BlockStackingSGN kernel for 8 Trainium2 NeuronCores.

Strategy: data-parallel over batch B=4096 -> 512 rows per core; all MLP
weights replicated. On-chip layout keeps activations transposed
([hidden -> partitions, batch -> free]) so every matmul streams the batch
through the PE with the weight stationary (bf16 operands, fp32 PSUM
accumulation). The three 256->1 output heads (clear / ontable / AonB) are
folded into one PSUM accumulation bank: each head's weight column is
embedded at output-row position r of a [128,128] stationary operand, so
all 80 output rows accumulate into a single [128, 512] bank and one
batched Sigmoid finishes the kernel. Elementwise work is spread across
the Scalar, Vector, and GpSimd engines to keep them all under the PE's
span; all 128-partition weights travel in one packed SBUF tile DMA'd in
parallel chunks.
"""

import sys

import numpy as np

sys.path.insert(0, "/opt/trn_rl_repo")

import concourse.bacc as bacc
import concourse.mybir as mybir
import concourse.tile as tile
from concourse.bass_utils import run_bass_kernel_spmd

dt = mybir.dt
AF = mybir.ActivationFunctionType
ALU = mybir.AluOpType

N = 8          # blocks
H = 256        # hidden
B = 4096       # batch
IN = 3 * N     # 24
NCORES = 8
BC = B // NCORES   # 512 batch rows per core
KT = H // 128      # k-tiles per 256-wide contraction
R = N * (N + 2)    # 80 output rows per batch element

F32 = dt.float32
BF16 = dt.bfloat16
W = BC

_CACHE = {}


def _wb_layout():
    """Column layout of the packed [128, ncols] bf16 weight tile.
    Order doubles as DMA arrival order: block-0 weights first."""
    keys = []
    for n in range(N):
        for k in range(KT):
            keys.append(("oW1", n, k))
            keys.append(("oW2", n, k))
    for nm in ("cW0", "cW1", "tW0", "tW1"):
        for k in range(KT):
            keys.append((nm, k))
    for nm in ("aW0l", "aW0r", "aW1"):
        for k in range(KT):
            keys.append((nm, k))
    for t_ in range(3):
        for k in range(KT):
            for p in range(2):
                keys.append(("w2e", t_, k, p))
    off = {}
    col = 0
    for key in keys:
        off[key] = col
        col += H
    return off, col


WB_OFF, WB_COLS = _wb_layout()

# bias tile column layout ([128, nb] fp32)
def _bias_layout():
    keys = []
    for n in range(N):
        for nm in ("ob0", "ob1", "ob2"):
            for m in range(KT):
                keys.append((nm, n, m))
    for nm in ("cb0", "cb1", "tb0", "tb1", "ab0", "ab1"):
        for m in range(KT):
            keys.append((nm, m))
    keys.append(("finb",))
    return {k: i for i, k in enumerate(keys)}, len(keys)


BIAS_OFF, BIAS_COLS = _bias_layout()

N_DMA_CHUNKS = 8


def _build():
    nc = bacc.Bacc("TRN2", target_bir_lowering=False, debug=False, num_devices=NCORES)

    d_xT = nc.dram_tensor("xT", [IN, BC], BF16, kind="ExternalInput")
    d_wb = nc.dram_tensor("wb", [128, WB_COLS], BF16, kind="ExternalInput")
    d_ow0 = nc.dram_tensor("ow0", [IN, N * H], BF16, kind="ExternalInput")
    d_bias = nc.dram_tensor("bias", [128, BIAS_COLS], F32, kind="ExternalInput")
    d_out = nc.dram_tensor("outT", [R, BC], F32, kind="ExternalOutput")

    with tile.TileContext(nc) as tc:
        with (
            tc.tile_pool(name="w", bufs=1) as wp,
            tc.tile_pool(name="act", bufs=1) as ap,
            tc.tile_pool(name="wk", bufs=4) as wk,
            tc.tile_pool(name="ps", bufs=6, space="PSUM") as ps,
            tc.tile_pool(name="finp", bufs=1, space="PSUM") as fp,
        ):
            xT = wp.tile([IN, BC], BF16, tag="xT")
            nc.sync.dma_start(xT[:], d_xT[:])
            bias = wp.tile([128, BIAS_COLS], F32, tag="bias")
            nc.sync.dma_start(bias[:], d_bias[:])
            ow0 = wp.tile([IN, N * H], BF16, tag="ow0")
            nc.gpsimd.dma_start(ow0[:], d_ow0[:])

            wb = wp.tile([128, WB_COLS], BF16, tag="wb")
            chunk = (WB_COLS + N_DMA_CHUNKS - 1) // N_DMA_CHUNKS
            for c in range(N_DMA_CHUNKS):
                eng = nc.sync if c % 2 == 0 else nc.gpsimd
                sl = slice(c * chunk, min((c + 1) * chunk, WB_COLS))
                eng.dma_start(wb[:, sl], d_wb[:, sl])

            def wslice(*key):
                o = WB_OFF[key]
                return lambda m: wb[:, o + m * 128 : o + (m + 1) * 128]

            def bcol(*key):
                i = BIAS_OFF[key]
                return bias[:, i : i + 1]

            fin = fp.tile([128, BC], F32, tag="fin")
            n_fin = 2 * KT * N + KT * N * N
            fin_ct = [0]

            def fin_mm(t_, k, r, rhs):
                first = fin_ct[0] == 0
                fin_ct[0] += 1
                last = fin_ct[0] == n_fin
                p = r % 2
                o = WB_OFF[("w2e", t_, k, p)]
                lhsT = wb[:, o + 128 + p - r : o + 256 + p - r]
                nc.tensor.matmul(fin[:], lhsT, rhs, start=first, stop=last)

            # weighted round-robin of PSUM evacuations: 3 ACT : 2 DVE
            evac_ct = [0]
            EV_PAT = "AADAADA"

            def evac(out_ap, psum_ap, bias_ap, relu):
                e = EV_PAT[evac_ct[0] % len(EV_PAT)]
                evac_ct[0] += 1
                if e == "A":
                    func = AF.Relu if relu else (AF.Identity if bias_ap is not None else AF.Copy)
                    if bias_ap is not None:
                        nc.scalar.activation(out_ap, psum_ap, func, bias=bias_ap)
                    else:
                        nc.scalar.activation(out_ap, psum_ap, func)
                else:
                    if relu:
                        b = bias_ap if bias_ap is not None else 0.0
                        nc.vector.tensor_scalar(out_ap, psum_ap, b, 0.0, ALU.add, ALU.max)
                    elif bias_ap is not None:
                        nc.vector.tensor_scalar(out_ap, psum_ap, bias_ap, None, ALU.add)
                    else:
                        nc.vector.tensor_copy(out_ap, psum_ap)

            def layer(wgt, bias_m, in_tile, in_parts, relu, out_pool, out_tag):
                """One 256-out layer -> [128, KT*W] tile.
                wgt(k) -> fn m -> lhsT AP; in_parts: list of rhs APs (k-tiles)."""
                out = out_pool.tile([128, KT * W], BF16, tag=out_tag)
                if in_parts is None:
                    in_parts = [in_tile[:, k * W : (k + 1) * W] for k in range(KT)]
                for m in range(KT):
                    pst = ps.tile([128, BC], F32, tag="ps")
                    for ki, rhs in enumerate(in_parts):
                        nc.tensor.matmul(pst[:], wgt(ki)(m), rhs,
                                         start=(ki == 0), stop=(ki == len(in_parts) - 1))
                    evac(out[:, m * W : (m + 1) * W], pst[:],
                         bias_m(m) if bias_m else None, relu)
                return out

            # ---- object encoders -> enc[n] [128, 2W] (persistent) ----
            enc = []
            for n in range(N):
                ow0_l = lambda n=n: (lambda ki: (lambda m: ow0[:, n * H + m * 128 : n * H + (m + 1) * 128]))
                h0 = layer(ow0_l(), lambda m, n=n: bcol("ob0", n, m), None, [xT[:]], True, wk, "h")
                h1 = layer(lambda ki, n=n: wslice("oW1", n, ki), lambda m, n=n: bcol("ob1", n, m),
                           h0, None, True, wk, "h")
                e = layer(lambda ki, n=n: wslice("oW2", n, ki), lambda m, n=n: bcol("ob2", n, m),
                          h1, None, False, ap, f"enc_{n}")
                enc.append(e)

            # ---- clear / ontable predicates -> head rows i*10+8 / i*10+9 ----
            for n in range(N):
                for (w0nm, w1nm, b0nm, b1nm, w2idx, r) in (
                    ("cW0", "cW1", "cb0", "cb1", 0, n * 10 + 8),
                    ("tW0", "tW1", "tb0", "tb1", 1, n * 10 + 9),
                ):
                    y0 = layer(lambda ki, w0nm=w0nm: wslice(w0nm, ki),
                               lambda m, b0nm=b0nm: bcol(b0nm, m), enc[n], None, True, wk, "h")
                    y1 = layer(lambda ki, w1nm=w1nm: wslice(w1nm, ki),
                               lambda m, b1nm=b1nm: bcol(b1nm, m), y0, None, True, wk, "h")
                    for k in range(KT):
                        fin_mm(w2idx, k, r, y1[:, k * W : (k + 1) * W])

            # ---- AonB pair-input halves (bias a_b0 folded into left) ----
            al = [layer(lambda ki: wslice("aW0l", ki), lambda m: bcol("ab0", m),
                        enc[n], None, False, ap, f"al_{n}") for n in range(N)]
            ar = [layer(lambda ki: wslice("aW0r", ki), None,
                        enc[n], None, False, ap, f"ar_{n}") for n in range(N)]

            # ---- all (i, j) pairs, ordered round-robin across the three
            # 32-row PSUM partition groups so packed head matmuls overlap ----
            zero = wp.tile([128, KT * W], BF16, tag="zero")
            nc.gpsimd.memset(zero[:], 0.0)
            buckets = [[], [], []]
            for i in range(N):
                for j in range(N):
                    buckets[(i * 10 + j) // 32].append((i, j))
            order = []
            bi = 0
            while any(buckets):
                if buckets[bi % 3]:
                    order.append(buckets[bi % 3].pop(0))
                bi += 1
            for (i, j) in order:
                r = i * 10 + j
                phs = wk.tile([128, KT * W], BF16, tag="phs")
                nc.vector.tensor_tensor(phs[:], al[i][:], ar[j][:], ALU.add)
                ph = wk.tile([128, KT * W], BF16, tag="ph")
                nc.vector.tensor_tensor(ph[:], phs[:], zero[:], ALU.max)
                y = layer(lambda ki: wslice("aW1", ki), lambda m: bcol("ab1", m),
                          ph, None, True, wk, "y")
                for k in range(KT):
                    fin_mm(2, k, r, y[:, k * W : (k + 1) * W])

            assert fin_ct[0] == n_fin

            # ---- batched sigmoid over all 80 head rows + store ----
            outT = wk.tile([128, BC], F32, tag="outT")
            nc.scalar.activation(outT[:], fin[:], AF.Sigmoid, bias=bcol("finb"))
            nc.sync.dma_start(d_out[:], outT[:R, :])

    nc.compile()
    return nc


def _prep_inputs(inputs):
    import ml_dtypes

    bf = ml_dtypes.bfloat16
    f32a = lambda a: np.asarray(a, dtype=np.float32)

    wbv = np.zeros((128, WB_COLS), bf)

    def put(key, arr):  # arr: [128, 256] fp32
        o = WB_OFF[key]
        wbv[:, o : o + H] = arr.astype(bf)

    oW1 = f32a(inputs["o_W1"])
    oW2 = f32a(inputs["o_W2"])
    for n in range(N):
        for k in range(KT):
            put(("oW1", n, k), oW1[n, k * 128 : (k + 1) * 128])
            put(("oW2", n, k), oW2[n, k * 128 : (k + 1) * 128])
    for nm, src in (("cW0", "c_W0"), ("cW1", "c_W1"), ("tW0", "t_W0"), ("tW1", "t_W1")):
        a = f32a(inputs[src])
        for k in range(KT):
            put((nm, k), a[k * 128 : (k + 1) * 128])
    aW0 = f32a(inputs["a_W0"])
    for k in range(KT):
        put(("aW0l", k), aW0[k * 128 : (k + 1) * 128])
        put(("aW0r", k), aW0[H + k * 128 : H + (k + 1) * 128])
    aW1 = f32a(inputs["a_W1"])
    for k in range(KT):
        put(("aW1", k), aW1[k * 128 : (k + 1) * 128])
    for t_, src2 in enumerate(("c_W2", "t_W2", "a_W2")):
        w2 = f32a(inputs[src2])[:, 0].astype(bf)
        for k in range(KT):
            for p in range(2):
                o = WB_OFF[("w2e", t_, k, p)]
                wbv[:, o + 128 + p] = w2[k * 128 : (k + 1) * 128]

    biasv = np.zeros((128, BIAS_COLS), np.float32)

    def putb(key, vec128):
        biasv[:, BIAS_OFF[key]] = vec128

    for n in range(N):
        for nm, src in (("ob0", "o_b0"), ("ob1", "o_b1"), ("ob2", "o_b2")):
            a = f32a(inputs[src])[n]
            for m in range(KT):
                putb((nm, n, m), a[m * 128 : (m + 1) * 128])
    for nm, src in (("cb0", "c_b0"), ("cb1", "c_b1"), ("tb0", "t_b0"),
                    ("tb1", "t_b1"), ("ab0", "a_b0"), ("ab1", "a_b1")):
        a = f32a(inputs[src])
        for m in range(KT):
            putb((nm, m), a[m * 128 : (m + 1) * 128])
    finb = np.zeros(128, np.float32)
    for i in range(N):
        finb[i * 10 : i * 10 + 8] = f32a(inputs["a_b2"])[0]
        finb[i * 10 + 8] = f32a(inputs["c_b2"])[0]
        finb[i * 10 + 9] = f32a(inputs["t_b2"])[0]
    putb(("finb",), finb)

    ow0v = np.zeros((IN, N * H), bf)
    oW0 = f32a(inputs["o_W0"])
    for n in range(N):
        ow0v[:, n * H : (n + 1) * H] = oW0[n].astype(bf)

    xT = np.ascontiguousarray(f32a(inputs["x"]).T)  # (24, 4096)
    common = {"wb": wbv, "ow0": ow0v, "bias": biasv}
    in_maps = []
    for c in range(NCORES):
        m = dict(common)
        m["xT"] = np.ascontiguousarray(xT[:, c * BC : (c + 1) * BC]).astype(bf)
        in_maps.append(m)
    return in_maps


def run(inputs, trace=False, **kw):
    if "nc" not in _CACHE:
        _CACHE["nc"] = _build()
    nc = _CACHE["nc"]
    in_maps = _prep_inputs(inputs)
    res = run_bass_kernel_spmd(nc, in_maps, list(range(NCORES)), trace=trace, **kw)
    out = np.concatenate([res.results[c]["outT"].T for c in range(NCORES)], axis=0)
    return out.astype(np.float32), res


def kernel(**inputs) -> np.ndarray:
    out, _ = run(inputs, trace=False)
    return out


# revision 7
# speedup vs baseline: 1.4547x; 1.0775x over previous
"""BlockStackingSGN kernel for 8 Trainium2 NeuronCores.

Strategy: data-parallel over batch B=4096 -> 512 rows per core; all MLP
weights replicated. On-chip layout keeps activations transposed
([hidden -> partitions, batch -> free]) so every matmul streams the batch
through the PE with the weight stationary (bf16 operands, fp32 PSUM
accumulation). The three 256->1 output heads (clear / ontable / AonB) are
folded into one PSUM accumulation bank: each head's weight column is
embedded at output-row position r of a [128,128] stationary operand, so
all 80 output rows accumulate into a single [128, 512] bank and one
batched Sigmoid finishes the kernel. Elementwise work is spread across
the Scalar, Vector, and GpSimd engines to keep them all under the PE's
span; all 128-partition weights travel in one packed SBUF tile DMA'd in
parallel chunks.
"""

import sys

import numpy as np

sys.path.insert(0, "/opt/trn_rl_repo")

import concourse.bacc as bacc
import concourse.mybir as mybir
import concourse.tile as tile
from concourse.bass_utils import run_bass_kernel_spmd

dt = mybir.dt
AF = mybir.ActivationFunctionType
ALU = mybir.AluOpType

N = 8          # blocks
H = 256        # hidden
B = 4096       # batch
IN = 3 * N     # 24
NCORES = 8
BC = B // NCORES   # 512 batch rows per core
KT = H // 128      # k-tiles per 256-wide contraction
R = N * (N + 2)    # 80 output rows per batch element

F32 = dt.float32
BF16 = dt.bfloat16
W = BC

_CACHE = {}


def _wb_layout():
    """Column layout of the packed [128, ncols] bf16 weight tile.
    Order doubles as DMA arrival order: block-0 weights first."""
    keys = []
    for n in range(N):
        for k in range(KT):
            keys.append(("oW1", n, k))
            keys.append(("oW2", n, k))
    for nm in ("cW0", "cW1", "tW0", "tW1"):
        for k in range(KT):
            keys.append((nm, k))
    for nm in ("aW0l", "aW0r", "aW1"):
        for k in range(KT):
            keys.append((nm, k))
    for t_ in range(3):
        for k in range(KT):
            for p in range(2):
                keys.append(("w2e", t_, k, p))
    off = {}
    col = 0
    for key in keys:
        off[key] = col
        col += H
    return off, col


WB_OFF, WB_COLS = _wb_layout()

# bias tile column layout ([128, nb] fp32)
def _bias_layout():
    keys = []
    for n in range(N):
        for nm in ("ob0", "ob1", "ob2"):
            for m in range(KT):
                keys.append((nm, n, m))
    for nm in ("cb0", "cb1", "tb0", "tb1", "ab0", "ab1"):
        for m in range(KT):
            keys.append((nm, m))
    keys.append(("finb",))
    return {k: i for i, k in enumerate(keys)}, len(keys)


BIAS_OFF, BIAS_COLS = _bias_layout()

N_DMA_CHUNKS = 8


def _build():
    nc = bacc.Bacc("TRN2", target_bir_lowering=False, debug=False, num_devices=NCORES)

    d_xT = nc.dram_tensor("xT", [IN, BC], BF16, kind="ExternalInput")
    d_wb = nc.dram_tensor("wb", [128, WB_COLS], BF16, kind="ExternalInput")
    d_ow0 = nc.dram_tensor("ow0", [IN, N * H], BF16, kind="ExternalInput")
    d_bias = nc.dram_tensor("bias", [128, BIAS_COLS], F32, kind="ExternalInput")
    d_out = nc.dram_tensor("outT", [R, BC], F32, kind="ExternalOutput")

    with tile.TileContext(nc) as tc:
        with (
            tc.tile_pool(name="w", bufs=1) as wp,
            tc.tile_pool(name="act", bufs=1) as ap,
            tc.tile_pool(name="wk", bufs=4) as wk,
            tc.tile_pool(name="ps", bufs=6, space="PSUM") as ps,
            tc.tile_pool(name="finp", bufs=1, space="PSUM") as fp,
        ):
            xT = wp.tile([IN, BC], BF16, tag="xT")
            nc.sync.dma_start(xT[:], d_xT[:])
            bias = wp.tile([128, BIAS_COLS], F32, tag="bias")
            nc.sync.dma_start(bias[:], d_bias[:])
            ow0 = wp.tile([IN, N * H], BF16, tag="ow0")
            nc.gpsimd.dma_start(ow0[:], d_ow0[:])

            wb = wp.tile([128, WB_COLS], BF16, tag="wb")
            chunk = (WB_COLS + N_DMA_CHUNKS - 1) // N_DMA_CHUNKS
            for c in range(N_DMA_CHUNKS):
                eng = nc.sync if c % 2 == 0 else nc.gpsimd
                sl = slice(c * chunk, min((c + 1) * chunk, WB_COLS))
                eng.dma_start(wb[:, sl], d_wb[:, sl])

            def wslice(*key):
                o = WB_OFF[key]
                return lambda m: wb[:, o + m * 128 : o + (m + 1) * 128]

            def bcol(*key):
                i = BIAS_OFF[key]
                return bias[:, i : i + 1]

            zero = wp.tile([128, KT * W], BF16, tag="zero")
            nc.gpsimd.memset(zero[:], 0.0)
            fin = fp.tile([128, BC], F32, tag="fin")
            n_fin = 2 * KT * N + KT * N * N
            fin_ct = [0]

            def fin_mm(t_, k, r, rhs):
                first = fin_ct[0] == 0
                fin_ct[0] += 1
                last = fin_ct[0] == n_fin
                p = r % 2
                o = WB_OFF[("w2e", t_, k, p)]
                lhsT = wb[:, o + 128 + p - r : o + 256 + p - r]
                nc.tensor.matmul(fin[:], lhsT, rhs, start=first, stop=last)

            # weighted round-robin of PSUM evacuations: 3 ACT : 2 DVE
            evac_ct = [0]
            EV_PAT = "AADAADA"

            def evac(out_ap, psum_ap, bias_ap, relu):
                e = EV_PAT[evac_ct[0] % len(EV_PAT)]
                evac_ct[0] += 1
                if e == "A":
                    func = AF.Relu if relu else (AF.Identity if bias_ap is not None else AF.Copy)
                    if bias_ap is not None:
                        nc.scalar.activation(out_ap, psum_ap, func, bias=bias_ap)
                    else:
                        nc.scalar.activation(out_ap, psum_ap, func)
                else:
                    if relu:
                        b = bias_ap if bias_ap is not None else 0.0
                        nc.vector.tensor_scalar(out_ap, psum_ap, b, 0.0, ALU.add, ALU.max)
                    elif bias_ap is not None:
                        nc.vector.tensor_scalar(out_ap, psum_ap, bias_ap, None, ALU.add)
                    else:
                        nc.vector.tensor_copy(out_ap, psum_ap)

            def layer(wgt, bias_m, in_tile, in_parts, relu, out_pool, out_tag):
                """One 256-out layer -> [128, KT*W] tile.
                wgt(k) -> fn m -> lhsT AP; in_parts: list of rhs APs (k-tiles)."""
                out = out_pool.tile([128, KT * W], BF16, tag=out_tag)
                if in_parts is None:
                    in_parts = [in_tile[:, k * W : (k + 1) * W] for k in range(KT)]
                for m in range(KT):
                    pst = ps.tile([128, BC], F32, tag="ps")
                    for ki, rhs in enumerate(in_parts):
                        nc.tensor.matmul(pst[:], wgt(ki)(m), rhs,
                                         start=(ki == 0), stop=(ki == len(in_parts) - 1))
                    evac(out[:, m * W : (m + 1) * W], pst[:],
                         bias_m(m) if bias_m else None, relu)
                return out

            # ---- object encoders -> enc[n] [128, 2W] (persistent) ----
            enc = []
            for n in range(N):
                ow0_l = lambda n=n: (lambda ki: (lambda m: ow0[:, n * H + m * 128 : n * H + (m + 1) * 128]))
                h0 = layer(ow0_l(), lambda m, n=n: bcol("ob0", n, m), None, [xT[:]], True, wk, "h")
                h1 = layer(lambda ki, n=n: wslice("oW1", n, ki), lambda m, n=n: bcol("ob1", n, m),
                           h0, None, True, wk, "h")
                e = layer(lambda ki, n=n: wslice("oW2", n, ki), lambda m, n=n: bcol("ob2", n, m),
                          h1, None, False, ap, f"enc_{n}")
                enc.append(e)

            # ---- AonB pair-input halves (bias a_b0 folded into left) ----
            al, ar = [], []
            for n in range(N):
                al.append(layer(lambda ki: wslice("aW0l", ki), lambda m: bcol("ab0", m),
                                enc[n], None, False, ap, f"al_{n}"))
                ar.append(layer(lambda ki: wslice("aW0r", ki), None,
                                enc[n], None, False, ap, f"ar_{n}"))

            # ---- clear / ontable predicate thunks (interleaved into pairs) ----
            def pred_thunk(n, w0nm, w1nm, b0nm, b1nm, w2idx, r):
                def go():
                    y0 = layer(lambda ki: wslice(w0nm, ki),
                               lambda m: bcol(b0nm, m), enc[n], None, True, wk, "h")
                    y1 = layer(lambda ki: wslice(w1nm, ki),
                               lambda m: bcol(b1nm, m), y0, None, True, wk, "h")
                    for k in range(KT):
                        fin_mm(w2idx, k, r, y1[:, k * W : (k + 1) * W])
                return go

            preds = []
            for n in range(N):
                preds.append(pred_thunk(n, "cW0", "cW1", "cb0", "cb1", 0, n * 10 + 8))
                preds.append(pred_thunk(n, "tW0", "tW1", "tb0", "tb1", 1, n * 10 + 9))

            # ---- all (i, j) pairs, ordered round-robin across the three
            # 32-row PSUM partition groups ----
            buckets = [[], [], []]
            for i in range(N):
                for j in range(N):
                    buckets[(i * 10 + j) // 32].append((i, j))
            order = []
            bi = 0
            while any(buckets):
                if buckets[bi % 3]:
                    order.append(buckets[bi % 3].pop(0))
                bi += 1
            for pi, (i, j) in enumerate(order):
                if pi % 4 == 0 and preds:
                    preds.pop(0)()
                r = i * 10 + j
                phs = wk.tile([128, KT * W], BF16, tag="phs")
                nc.vector.tensor_tensor(phs[:], al[i][:], ar[j][:], ALU.add)
                ph = wk.tile([128, KT * W], BF16, tag="ph")
                nc.vector.tensor_tensor(ph[:], phs[:], zero[:], ALU.max)
                y = layer(lambda ki: wslice("aW1", ki), lambda m: bcol("ab1", m),
                          ph, None, True, wk, "y")
                for k in range(KT):
                    fin_mm(2, k, r, y[:, k * W : (k + 1) * W])
            for t in preds:
                t()

            assert fin_ct[0] == n_fin

            # ---- batched sigmoid over all 80 head rows + store ----
            outT = wk.tile([128, BC], F32, tag="outT")
            nc.scalar.activation(outT[:], fin[:], AF.Sigmoid, bias=bcol("finb"))
            nc.sync.dma_start(d_out[:], outT[:R, :])

    nc.compile()
    return nc


def _prep_inputs(inputs):
    import ml_dtypes

    bf = ml_dtypes.bfloat16
    f32a = lambda a: np.asarray(a, dtype=np.float32)

    wbv = np.zeros((128, WB_COLS), bf)

    def put(key, arr):  # arr: [128, 256] fp32
        o = WB_OFF[key]
        wbv[:, o : o + H] = arr.astype(bf)

    oW1 = f32a(inputs["o_W1"])
    oW2 = f32a(inputs["o_W2"])
    for n in range(N):
        for k in range(KT):
            put(("oW1", n, k), oW1[n, k * 128 : (k + 1) * 128])
            put(("oW2", n, k), oW2[n, k * 128 : (k + 1) * 128])
    for nm, src in (("cW0", "c_W0"), ("cW1", "c_W1"), ("tW0", "t_W0"), ("tW1", "t_W1")):
        a = f32a(inputs[src])
        for k in range(KT):
            put((nm, k), a[k * 128 : (k + 1) * 128])
    aW0 = f32a(inputs["a_W0"])
    for k in range(KT):
        put(("aW0l", k), aW0[k * 128 : (k + 1) * 128])
        put(("aW0r", k), aW0[H + k * 128 : H + (k + 1) * 128])
    aW1 = f32a(inputs["a_W1"])
    for k in range(KT):
        put(("aW1", k), aW1[k * 128 : (k + 1) * 128])
    for t_, src2 in enumerate(("c_W2", "t_W2", "a_W2")):
        w2 = f32a(inputs[src2])[:, 0].astype(bf)
        for k in range(KT):
            for p in range(2):
                o = WB_OFF[("w2e", t_, k, p)]
                wbv[:, o + 128 + p] = w2[k * 128 : (k + 1) * 128]

    biasv = np.zeros((128, BIAS_COLS), np.float32)

    def putb(key, vec128):
        biasv[:, BIAS_OFF[key]] = vec128

    for n in range(N):
        for nm, src in (("ob0", "o_b0"), ("ob1", "o_b1"), ("ob2", "o_b2")):
            a = f32a(inputs[src])[n]
            for m in range(KT):
                putb((nm, n, m), a[m * 128 : (m + 1) * 128])
    for nm, src in (("cb0", "c_b0"), ("cb1", "c_b1"), ("tb0", "t_b0"),
                    ("tb1", "t_b1"), ("ab0", "a_b0"), ("ab1", "a_b1")):
        a = f32a(inputs[src])
        for m in range(KT):
            putb((nm, m), a[m * 128 : (m + 1) * 128])
    finb = np.zeros(128, np.float32)
    for i in range(N):
        finb[i * 10 : i * 10 + 8] = f32a(inputs["a_b2"])[0]
        finb[i * 10 + 8] = f32a(inputs["c_b2"])[0]
        finb[i * 10 + 9] = f32a(inputs["t_b2"])[0]
    putb(("finb",), finb)

    ow0v = np.zeros((IN, N * H), bf)
    oW0 = f32a(inputs["o_W0"])
    for n in range(N):
        ow0v[:, n * H : (n + 1) * H] = oW0[n].astype(bf)

    xT = np.ascontiguousarray(f32a(inputs["x"]).T)  # (24, 4096)
    common = {"wb": wbv, "ow0": ow0v, "bias": biasv}
    in_maps = []
    for c in range(NCORES):
        m = dict(common)
        m["xT"] = np.ascontiguousarray(xT[:, c * BC : (c + 1) * BC]).astype(bf)
        in_maps.append(m)
    return in_maps


def run(inputs, trace=False, **kw):
    if "nc" not in _CACHE:
        _CACHE["nc"] = _build()
    nc = _CACHE["nc"]
    in_maps = _prep_inputs(inputs)
    res = run_bass_kernel_spmd(nc, in_maps, list(range(NCORES)), trace=trace, **kw)
    out = np.concatenate([res.results[c]["outT"].T for c in range(NCORES)], axis=0)
    return out.astype(np.float32), res


def kernel(**inputs) -> np.ndarray:
    out, _ = run(inputs, trace=False)
    return out


# revision 8
# speedup vs baseline: 1.4664x; 1.0080x over previous
"""BlockStackingSGN kernel for 8 Trainium2 NeuronCores.

Strategy: data-parallel over batch B=4096 -> 512 rows per core; all MLP
weights replicated. On-chip layout keeps activations transposed
([hidden -> partitions, batch -> free]) so every matmul streams the batch
through the PE with the weight stationary (bf16 operands, fp32 PSUM
accumulation). The three 256->1 output heads (clear / ontable / AonB) are
folded into one PSUM accumulation bank: each head's weight column is
embedded at output-row position r of a [128,128] stationary operand, so
all 80 output rows accumulate into a single [128, 512] bank and one
batched Sigmoid finishes the kernel. Elementwise work is spread across
the Scalar, Vector, and GpSimd engines to keep them all under the PE's
span; all 128-partition weights travel in one packed SBUF tile DMA'd in
parallel chunks.
"""

import sys

import numpy as np

sys.path.insert(0, "/opt/trn_rl_repo")

import concourse.bacc as bacc
import concourse.mybir as mybir
import concourse.tile as tile
from concourse.bass_utils import run_bass_kernel_spmd

dt = mybir.dt
AF = mybir.ActivationFunctionType
ALU = mybir.AluOpType

N = 8          # blocks
H = 256        # hidden
B = 4096       # batch
IN = 3 * N     # 24
NCORES = 8
BC = B // NCORES   # 512 batch rows per core
KT = H // 128      # k-tiles per 256-wide contraction
R = N * (N + 2)    # 80 output rows per batch element

F32 = dt.float32
BF16 = dt.bfloat16
W = BC

_CACHE = {}


def _wb_layout():
    """Column layout of the packed [128, ncols] bf16 weight tile.
    Order doubles as DMA arrival order: block-0 weights first."""
    keys = []
    for n in range(N):
        for k in range(KT):
            keys.append(("oW1", n, k))
            keys.append(("oW2", n, k))
    for nm in ("cW0", "cW1", "tW0", "tW1"):
        for k in range(KT):
            keys.append((nm, k))
    for nm in ("aW0l", "aW0r", "aW1"):
        for k in range(KT):
            keys.append((nm, k))
    for t_ in range(3):
        for k in range(KT):
            for p in range(2):
                keys.append(("w2e", t_, k, p))
    off = {}
    col = 0
    for key in keys:
        off[key] = col
        col += H
    return off, col


WB_OFF, WB_COLS = _wb_layout()

# bias tile column layout ([128, nb] fp32)
def _bias_layout():
    keys = []
    for n in range(N):
        for nm in ("ob0", "ob1", "ob2"):
            for m in range(KT):
                keys.append((nm, n, m))
    for nm in ("cb0", "cb1", "tb0", "tb1", "ab0", "ab1"):
        for m in range(KT):
            keys.append((nm, m))
    keys.append(("finb",))
    return {k: i for i, k in enumerate(keys)}, len(keys)


BIAS_OFF, BIAS_COLS = _bias_layout()

N_DMA_CHUNKS = 8


def _build():
    nc = bacc.Bacc("TRN2", target_bir_lowering=False, debug=False, num_devices=NCORES)

    d_xT = nc.dram_tensor("xT", [IN, BC], BF16, kind="ExternalInput")
    d_wb = nc.dram_tensor("wb", [128, WB_COLS], BF16, kind="ExternalInput")
    d_ow0 = nc.dram_tensor("ow0", [IN, N * H], BF16, kind="ExternalInput")
    d_bias = nc.dram_tensor("bias", [128, BIAS_COLS], F32, kind="ExternalInput")
    d_out = nc.dram_tensor("outT", [R, BC], F32, kind="ExternalOutput")

    with tile.TileContext(nc) as tc:
        with (
            tc.tile_pool(name="w", bufs=1) as wp,
            tc.tile_pool(name="act", bufs=1) as ap,
            tc.tile_pool(name="wk", bufs=4) as wk,
            tc.tile_pool(name="ps", bufs=7, space="PSUM") as ps,
            tc.tile_pool(name="finp", bufs=1, space="PSUM") as fp,
        ):
            xT = wp.tile([IN, BC], BF16, tag="xT")
            nc.sync.dma_start(xT[:], d_xT[:])
            bias = wp.tile([128, BIAS_COLS], F32, tag="bias")
            nc.sync.dma_start(bias[:], d_bias[:])
            ow0 = wp.tile([IN, N * H], BF16, tag="ow0")
            nc.gpsimd.dma_start(ow0[:], d_ow0[:])

            wb = wp.tile([128, WB_COLS], BF16, tag="wb")
            chunk = (WB_COLS + N_DMA_CHUNKS - 1) // N_DMA_CHUNKS
            for c in range(N_DMA_CHUNKS):
                eng = nc.sync if c % 2 == 0 else nc.gpsimd
                sl = slice(c * chunk, min((c + 1) * chunk, WB_COLS))
                eng.dma_start(wb[:, sl], d_wb[:, sl])

            def wslice(*key):
                o = WB_OFF[key]
                return lambda m: wb[:, o + m * 128 : o + (m + 1) * 128]

            def bcol(*key):
                i = BIAS_OFF[key]
                return bias[:, i : i + 1]

            zero = wp.tile([128, KT * W], BF16, tag="zero")
            nc.gpsimd.memset(zero[:], 0.0)
            fin = fp.tile([128, BC], F32, tag="fin")
            n_fin = 2 * KT * N + KT * N * N
            fin_ct = [0]

            def fin_mm(t_, k, r, rhs):
                first = fin_ct[0] == 0
                fin_ct[0] += 1
                last = fin_ct[0] == n_fin
                p = r % 2
                o = WB_OFF[("w2e", t_, k, p)]
                lhsT = wb[:, o + 128 + p - r : o + 256 + p - r]
                nc.tensor.matmul(fin[:], lhsT, rhs, start=first, stop=last)

            # weighted round-robin of PSUM evacuations: 3 ACT : 2 DVE
            evac_ct = [0]
            EV_PAT = "AADAADA"

            def evac(out_ap, psum_ap, bias_ap, relu):
                e = EV_PAT[evac_ct[0] % len(EV_PAT)]
                evac_ct[0] += 1
                if e == "A":
                    func = AF.Relu if relu else (AF.Identity if bias_ap is not None else AF.Copy)
                    if bias_ap is not None:
                        nc.scalar.activation(out_ap, psum_ap, func, bias=bias_ap)
                    else:
                        nc.scalar.activation(out_ap, psum_ap, func)
                else:
                    if relu:
                        b = bias_ap if bias_ap is not None else 0.0
                        nc.vector.tensor_scalar(out_ap, psum_ap, b, 0.0, ALU.add, ALU.max)
                    elif bias_ap is not None:
                        nc.vector.tensor_scalar(out_ap, psum_ap, bias_ap, None, ALU.add)
                    else:
                        nc.vector.tensor_copy(out_ap, psum_ap)

            def layer(wgt, bias_m, in_tile, in_parts, relu, out_pool, out_tag):
                """One 256-out layer -> [128, KT*W] tile.
                wgt(k) -> fn m -> lhsT AP; in_parts: list of rhs APs (k-tiles)."""
                out = out_pool.tile([128, KT * W], BF16, tag=out_tag)
                if in_parts is None:
                    in_parts = [in_tile[:, k * W : (k + 1) * W] for k in range(KT)]
                for m in range(KT):
                    pst = ps.tile([128, BC], F32, tag="ps")
                    for ki, rhs in enumerate(in_parts):
                        nc.tensor.matmul(pst[:], wgt(ki)(m), rhs,
                                         start=(ki == 0), stop=(ki == len(in_parts) - 1))
                    evac(out[:, m * W : (m + 1) * W], pst[:],
                         bias_m(m) if bias_m else None, relu)
                return out

            # ---- object encoders -> enc[n] [128, 2W] (persistent) ----
            enc = []
            for n in range(N):
                ow0_l = lambda n=n: (lambda ki: (lambda m: ow0[:, n * H + m * 128 : n * H + (m + 1) * 128]))
                h0 = layer(ow0_l(), lambda m, n=n: bcol("ob0", n, m), None, [xT[:]], True, wk, "h")
                h1 = layer(lambda ki, n=n: wslice("oW1", n, ki), lambda m, n=n: bcol("ob1", n, m),
                           h0, None, True, wk, "h")
                e = layer(lambda ki, n=n: wslice("oW2", n, ki), lambda m, n=n: bcol("ob2", n, m),
                          h1, None, False, ap, f"enc_{n}")
                enc.append(e)

            # ---- AonB pair-input halves (bias a_b0 folded into left) ----
            al, ar = [], []
            for n in range(N):
                al.append(layer(lambda ki: wslice("aW0l", ki), lambda m: bcol("ab0", m),
                                enc[n], None, False, ap, f"al_{n}"))
                ar.append(layer(lambda ki: wslice("aW0r", ki), None,
                                enc[n], None, False, ap, f"ar_{n}"))

            # ---- clear / ontable predicate thunks (interleaved into pairs) ----
            def pred_thunk(n, w0nm, w1nm, b0nm, b1nm, w2idx, r):
                def go():
                    y0 = layer(lambda ki: wslice(w0nm, ki),
                               lambda m: bcol(b0nm, m), enc[n], None, True, wk, "h")
                    y1 = layer(lambda ki: wslice(w1nm, ki),
                               lambda m: bcol(b1nm, m), y0, None, True, wk, "h")
                    for k in range(KT):
                        fin_mm(w2idx, k, r, y1[:, k * W : (k + 1) * W])
                return go

            preds = []
            for n in range(N):
                preds.append(pred_thunk(n, "cW0", "cW1", "cb0", "cb1", 0, n * 10 + 8))
                preds.append(pred_thunk(n, "tW0", "tW1", "tb0", "tb1", 1, n * 10 + 9))

            # ---- all (i, j) pairs, ordered round-robin across the three
            # 32-row PSUM partition groups ----
            buckets = [[], [], []]
            for i in range(N):
                for j in range(N):
                    buckets[(i * 10 + j) // 32].append((i, j))
            order = []
            bi = 0
            while any(buckets):
                if buckets[bi % 3]:
                    order.append(buckets[bi % 3].pop(0))
                bi += 1
            for pi, (i, j) in enumerate(order):
                if pi % 4 == 0 and preds:
                    preds.pop(0)()
                r = i * 10 + j
                phs = wk.tile([128, KT * W], BF16, tag="phs")
                nc.vector.tensor_tensor(phs[:], al[i][:], ar[j][:], ALU.add)
                ph = wk.tile([128, KT * W], BF16, tag="ph")
                nc.vector.tensor_tensor(ph[:], phs[:], zero[:], ALU.max)
                y = layer(lambda ki: wslice("aW1", ki), lambda m: bcol("ab1", m),
                          ph, None, True, wk, "y")
                for k in range(KT):
                    fin_mm(2, k, r, y[:, k * W : (k + 1) * W])
            for t in preds:
                t()

            assert fin_ct[0] == n_fin

            # ---- batched sigmoid over all 80 head rows + store ----
            outT = wk.tile([128, BC], F32, tag="outT")
            nc.scalar.activation(outT[:], fin[:], AF.Sigmoid, bias=bcol("finb"))
            nc.sync.dma_start(d_out[:], outT[:R, :])

    nc.compile()
    return nc


def _prep_inputs(inputs):
    import ml_dtypes

    bf = ml_dtypes.bfloat16
    f32a = lambda a: np.asarray(a, dtype=np.float32)

    wbv = np.zeros((128, WB_COLS), bf)

    def put(key, arr):  # arr: [128, 256] fp32
        o = WB_OFF[key]
        wbv[:, o : o + H] = arr.astype(bf)

    oW1 = f32a(inputs["o_W1"])
    oW2 = f32a(inputs["o_W2"])
    for n in range(N):
        for k in range(KT):
            put(("oW1", n, k), oW1[n, k * 128 : (k + 1) * 128])
            put(("oW2", n, k), oW2[n, k * 128 : (k + 1) * 128])
    for nm, src in (("cW0", "c_W0"), ("cW1", "c_W1"), ("tW0", "t_W0"), ("tW1", "t_W1")):
        a = f32a(inputs[src])
        for k in range(KT):
            put((nm, k), a[k * 128 : (k + 1) * 128])
    aW0 = f32a(inputs["a_W0"])
    for k in range(KT):
        put(("aW0l", k), aW0[k * 128 : (k + 1) * 128])
        put(("aW0r", k), aW0[H + k * 128 : H + (k + 1) * 128])
    aW1 = f32a(inputs["a_W1"])
    for k in range(KT):
        put(("aW1", k), aW1[k * 128 : (k + 1) * 128])
    for t_, src2 in enumerate(("c_W2", "t_W2", "a_W2")):
        w2 = f32a(inputs[src2])[:, 0].astype(bf)
        for k in range(KT):
            for p in range(2):
                o = WB_OFF[("w2e", t_, k, p)]
                wbv[:, o + 128 + p] = w2[k * 128 : (k + 1) * 128]

    biasv = np.zeros((128, BIAS_COLS), np.float32)

    def putb(key, vec128):
        biasv[:, BIAS_OFF[key]] = vec128

    for n in range(N):
        for nm, src in (("ob0", "o_b0"), ("ob1", "o_b1"), ("ob2", "o_b2")):
            a = f32a(inputs[src])[n]
            for m in range(KT):
                putb((nm, n, m), a[m * 128 : (m + 1) * 128])
    for nm, src in (("cb0", "c_b0"), ("cb1", "c_b1"), ("tb0", "t_b0"),
                    ("tb1", "t_b1"), ("ab0", "a_b0"), ("ab1", "a_b1")):
        a = f32a(inputs[src])
        for m in range(KT):
            putb((nm, m), a[m * 128 : (m + 1) * 128])
    finb = np.zeros(128, np.float32)
    for i in range(N):
        finb[i * 10 : i * 10 + 8] = f32a(inputs["a_b2"])[0]
        finb[i * 10 + 8] = f32a(inputs["c_b2"])[0]
        finb[i * 10 + 9] = f32a(inputs["t_b2"])[0]
    putb(("finb",), finb)

    ow0v = np.zeros((IN, N * H), bf)
    oW0 = f32a(inputs["o_W0"])
    for n in range(N):
        ow0v[:, n * H : (n + 1) * H] = oW0[n].astype(bf)

    xT = np.ascontiguousarray(f32a(inputs["x"]).T)  # (24, 4096)
    common = {"wb": wbv, "ow0": ow0v, "bias": biasv}
    in_maps = []
    for c in range(NCORES):
        m = dict(common)
        m["xT"] = np.ascontiguousarray(xT[:, c * BC : (c + 1) * BC]).astype(bf)
        in_maps.append(m)
    return in_maps


def run(inputs, trace=False, **kw):
    if "nc" not in _CACHE:
        _CACHE["nc"] = _build()
    nc = _CACHE["nc"]
    in_maps = _prep_inputs(inputs)
    res = run_bass_kernel_spmd(nc, in_maps, list(range(NCORES)), trace=trace, **kw)
    out = np.concatenate([res.results[c]["outT"].T for c in range(NCORES)], axis=0)
    return out.astype(np.float32), res


def kernel(**inputs) -> np.ndarray:
    out, _ = run(inputs, trace=False)
    return out


# revision 9
# speedup vs baseline: 1.5065x; 1.0274x over previous
"""BlockStackingSGN kernel for 8 Trainium2 NeuronCores.

Strategy: data-parallel over batch B=4096 -> 512 rows per core; all MLP
weights replicated. On-chip layout keeps activations transposed
([hidden -> partitions, batch -> free]) so every matmul streams the batch
through the PE with the weight stationary (bf16 operands, fp32 PSUM
accumulation). The three 256->1 output heads (clear / ontable / AonB) are
folded into one PSUM accumulation bank: each head's weight column is
embedded at output-row position r of a [128,128] stationary operand, so
all 80 output rows accumulate into a single [128, 512] bank and one
batched Sigmoid finishes the kernel. Elementwise work is spread across
the Scalar, Vector, and GpSimd engines to keep them all under the PE's
span; all 128-partition weights travel in one packed SBUF tile DMA'd in
parallel chunks.
"""

import sys

import numpy as np

sys.path.insert(0, "/opt/trn_rl_repo")

import concourse.bacc as bacc
import concourse.mybir as mybir
import concourse.tile as tile
from concourse.bass_utils import run_bass_kernel_spmd

dt = mybir.dt
AF = mybir.ActivationFunctionType
ALU = mybir.AluOpType

N = 8          # blocks
H = 256        # hidden
B = 4096       # batch
IN = 3 * N     # 24
NCORES = 8
BC = B // NCORES   # 512 batch rows per core
KT = H // 128      # k-tiles per 256-wide contraction
R = N * (N + 2)    # 80 output rows per batch element

F32 = dt.float32
BF16 = dt.bfloat16
W = BC

_CACHE = {}


def _wb_layout():
    """Column layout of the packed [128, ncols] bf16 weight tile.
    Order doubles as DMA arrival order: block-0 weights first."""
    keys = []
    for n in range(N):
        for k in range(KT):
            keys.append(("oW1", n, k))
            keys.append(("oW2", n, k))
    for nm in ("cW0", "cW1", "tW0", "tW1"):
        for k in range(KT):
            keys.append((nm, k))
    for nm in ("aW0l", "aW0r", "aW1"):
        for k in range(KT):
            keys.append((nm, k))
    for t_ in range(3):
        for k in range(KT):
            for p in range(2):
                keys.append(("w2e", t_, k, p))
    off = {}
    col = 0
    for key in keys:
        off[key] = col
        col += H
    return off, col


WB_OFF, WB_COLS = _wb_layout()

# bias tile column layout ([128, nb] fp32)
def _bias_layout():
    keys = []
    for n in range(N):
        for nm in ("ob0", "ob1", "ob2"):
            for m in range(KT):
                keys.append((nm, n, m))
    for nm in ("cb0", "cb1", "tb0", "tb1", "ab0", "ab1"):
        for m in range(KT):
            keys.append((nm, m))
    keys.append(("finb",))
    return {k: i for i, k in enumerate(keys)}, len(keys)


BIAS_OFF, BIAS_COLS = _bias_layout()

N_DMA_CHUNKS = 8


def _build():
    nc = bacc.Bacc("TRN2", target_bir_lowering=False, debug=False, num_devices=NCORES)

    d_xT = nc.dram_tensor("xT", [IN, BC], BF16, kind="ExternalInput")
    d_wb = nc.dram_tensor("wb", [128, WB_COLS], BF16, kind="ExternalInput")
    d_ow0 = nc.dram_tensor("ow0", [IN, N * H], BF16, kind="ExternalInput")
    d_bias = nc.dram_tensor("bias", [128, BIAS_COLS], F32, kind="ExternalInput")
    d_out = nc.dram_tensor("outT", [R, BC], F32, kind="ExternalOutput")

    with tile.TileContext(nc) as tc:
        with (
            tc.tile_pool(name="w", bufs=1) as wp,
            tc.tile_pool(name="act", bufs=1) as ap,
            tc.tile_pool(name="wk", bufs=4) as wk,
            tc.tile_pool(name="ps", bufs=7, space="PSUM") as ps,
            tc.tile_pool(name="finp", bufs=1, space="PSUM") as fp,
        ):
            xT = wp.tile([IN, BC], BF16, tag="xT")
            nc.sync.dma_start(xT[:], d_xT[:])
            ow0 = wp.tile([IN, N * H], BF16, tag="ow0")
            nc.gpsimd.dma_start(ow0[:], d_ow0[:])
            bias = wp.tile([128, BIAS_COLS], F32, tag="bias")
            nc.sync.dma_start(bias[:], d_bias[:])

            wb = wp.tile([128, WB_COLS], BF16, tag="wb")
            chunk = (WB_COLS + N_DMA_CHUNKS - 1) // N_DMA_CHUNKS
            for c in range(N_DMA_CHUNKS):
                eng = nc.gpsimd if c % 2 == 0 else nc.sync
                sl = slice(c * chunk, min((c + 1) * chunk, WB_COLS))
                eng.dma_start(wb[:, sl], d_wb[:, sl])

            def wslice(*key):
                o = WB_OFF[key]
                return lambda m: wb[:, o + m * 128 : o + (m + 1) * 128]

            def bcol(*key):
                i = BIAS_OFF[key]
                return bias[:, i : i + 1]

            fin = fp.tile([128, BC], F32, tag="fin")
            n_fin = 2 * KT * N + KT * N * N
            fin_ct = [0]

            def fin_mm(t_, k, r, rhs):
                first = fin_ct[0] == 0
                fin_ct[0] += 1
                last = fin_ct[0] == n_fin
                p = r % 2
                o = WB_OFF[("w2e", t_, k, p)]
                lhsT = wb[:, o + 128 + p - r : o + 256 + p - r]
                nc.tensor.matmul(fin[:], lhsT, rhs, start=first, stop=last)

            # weighted round-robin of PSUM evacuations: 3 ACT : 2 DVE
            evac_ct = [0]
            EV_PAT = "AADAADA"

            def evac(out_ap, psum_ap, bias_ap, relu):
                e = EV_PAT[evac_ct[0] % len(EV_PAT)]
                evac_ct[0] += 1
                if e == "A":
                    func = AF.Relu if relu else (AF.Identity if bias_ap is not None else AF.Copy)
                    if bias_ap is not None:
                        nc.scalar.activation(out_ap, psum_ap, func, bias=bias_ap)
                    else:
                        nc.scalar.activation(out_ap, psum_ap, func)
                else:
                    if relu:
                        b = bias_ap if bias_ap is not None else 0.0
                        nc.vector.tensor_scalar(out_ap, psum_ap, b, 0.0, ALU.add, ALU.max)
                    elif bias_ap is not None:
                        nc.vector.tensor_scalar(out_ap, psum_ap, bias_ap, None, ALU.add)
                    else:
                        nc.vector.tensor_copy(out_ap, psum_ap)

            def layer(wgt, bias_m, in_tile, in_parts, relu, out_pool, out_tag):
                """One 256-out layer -> [128, KT*W] tile.
                wgt(k) -> fn m -> lhsT AP; in_parts: list of rhs APs (k-tiles)."""
                out = out_pool.tile([128, KT * W], BF16, tag=out_tag)
                if in_parts is None:
                    in_parts = [in_tile[:, k * W : (k + 1) * W] for k in range(KT)]
                for m in range(KT):
                    pst = ps.tile([128, BC], F32, tag="ps")
                    for ki, rhs in enumerate(in_parts):
                        nc.tensor.matmul(pst[:], wgt(ki)(m), rhs,
                                         start=(ki == 0), stop=(ki == len(in_parts) - 1))
                    evac(out[:, m * W : (m + 1) * W], pst[:],
                         bias_m(m) if bias_m else None, relu)
                return out

            # ---- object encoders -> enc[n] [128, 2W] (persistent) ----
            enc = []
            for n in range(N):
                ow0_l = lambda n=n: (lambda ki: (lambda m: ow0[:, n * H + m * 128 : n * H + (m + 1) * 128]))
                h0 = layer(ow0_l(), lambda m, n=n: bcol("ob0", n, m), None, [xT[:]], True, wk, "h")
                h1 = layer(lambda ki, n=n: wslice("oW1", n, ki), lambda m, n=n: bcol("ob1", n, m),
                           h0, None, True, wk, "h")
                e = layer(lambda ki, n=n: wslice("oW2", n, ki), lambda m, n=n: bcol("ob2", n, m),
                          h1, None, False, ap, f"enc_{n}")
                enc.append(e)

            # ---- AonB pair-input halves (bias a_b0 folded into left) ----
            al, ar = [], []
            for n in range(N):
                al.append(layer(lambda ki: wslice("aW0l", ki), lambda m: bcol("ab0", m),
                                enc[n], None, False, ap, f"al_{n}"))
                ar.append(layer(lambda ki: wslice("aW0r", ki), None,
                                enc[n], None, False, ap, f"ar_{n}"))

            # ---- clear / ontable predicate thunks (interleaved into pairs) ----
            def pred_thunk(n, w0nm, w1nm, b0nm, b1nm, w2idx, r):
                def go():
                    y0 = layer(lambda ki: wslice(w0nm, ki),
                               lambda m: bcol(b0nm, m), enc[n], None, True, wk, "h")
                    y1 = layer(lambda ki: wslice(w1nm, ki),
                               lambda m: bcol(b1nm, m), y0, None, True, wk, "h")
                    for k in range(KT):
                        fin_mm(w2idx, k, r, y1[:, k * W : (k + 1) * W])
                return go

            preds = []
            for n in range(N):
                preds.append(pred_thunk(n, "cW0", "cW1", "cb0", "cb1", 0, n * 10 + 8))
                preds.append(pred_thunk(n, "tW0", "tW1", "tb0", "tb1", 1, n * 10 + 9))

            # ---- all (i, j) pairs, ordered round-robin across the three
            # 32-row PSUM partition groups ----
            zero = wp.tile([128, KT * W], BF16, tag="zero")
            nc.gpsimd.memset(zero[:], 0.0)
            buckets = [[], [], []]
            for i in range(N):
                for j in range(N):
                    buckets[(i * 10 + j) // 32].append((i, j))
            order = []
            bi = 0
            while any(buckets):
                if buckets[bi % 3]:
                    order.append(buckets[bi % 3].pop(0))
                bi += 1
            for pi, (i, j) in enumerate(order):
                if pi % 4 == 0 and preds:
                    preds.pop(0)()
                r = i * 10 + j
                phs = wk.tile([128, KT * W], BF16, tag="phs")
                nc.vector.tensor_tensor(phs[:], al[i][:], ar[j][:], ALU.add)
                ph = wk.tile([128, KT * W], BF16, tag="ph")
                nc.vector.tensor_tensor(ph[:], phs[:], zero[:], ALU.max)
                y = layer(lambda ki: wslice("aW1", ki), lambda m: bcol("ab1", m),
                          ph, None, True, wk, "y")
                for k in range(KT):
                    fin_mm(2, k, r, y[:, k * W : (k + 1) * W])
            for t in preds:
                t()

            assert fin_ct[0] == n_fin

            # ---- batched sigmoid over all 80 head rows + store ----
            outT = wk.tile([128, BC], F32, tag="outT")
            nc.scalar.activation(outT[:], fin[:], AF.Sigmoid, bias=bcol("finb"))
            nc.sync.dma_start(d_out[:], outT[:R, :])

    nc.compile()
    return nc


def _prep_inputs(inputs):
    import ml_dtypes

    bf = ml_dtypes.bfloat16
    f32a = lambda a: np.asarray(a, dtype=np.float32)

    wbv = np.zeros((128, WB_COLS), bf)

    def put(key, arr):  # arr: [128, 256] fp32
        o = WB_OFF[key]
        wbv[:, o : o + H] = arr.astype(bf)

    oW1 = f32a(inputs["o_W1"])
    oW2 = f32a(inputs["o_W2"])
    for n in range(N):
        for k in range(KT):
            put(("oW1", n, k), oW1[n, k * 128 : (k + 1) * 128])
            put(("oW2", n, k), oW2[n, k * 128 : (k + 1) * 128])
    for nm, src in (("cW0", "c_W0"), ("cW1", "c_W1"), ("tW0", "t_W0"), ("tW1", "t_W1")):
        a = f32a(inputs[src])
        for k in range(KT):
            put((nm, k), a[k * 128 : (k + 1) * 128])
    aW0 = f32a(inputs["a_W0"])
    for k in range(KT):
        put(("aW0l", k), aW0[k * 128 : (k + 1) * 128])
        put(("aW0r", k), aW0[H + k * 128 : H + (k + 1) * 128])
    aW1 = f32a(inputs["a_W1"])
    for k in range(KT):
        put(("aW1", k), aW1[k * 128 : (k + 1) * 128])
    for t_, src2 in enumerate(("c_W2", "t_W2", "a_W2")):
        w2 = f32a(inputs[src2])[:, 0].astype(bf)
        for k in range(KT):
            for p in range(2):
                o = WB_OFF[("w2e", t_, k, p)]
                wbv[:, o + 128 + p] = w2[k * 128 : (k + 1) * 128]

    biasv = np.zeros((128, BIAS_COLS), np.float32)

    def putb(key, vec128):
        biasv[:, BIAS_OFF[key]] = vec128

    for n in range(N):
        for nm, src in (("ob0", "o_b0"), ("ob1", "o_b1"), ("ob2", "o_b2")):
            a = f32a(inputs[src])[n]
            for m in range(KT):
                putb((nm, n, m), a[m * 128 : (m + 1) * 128])
    for nm, src in (("cb0", "c_b0"), ("cb1", "c_b1"), ("tb0", "t_b0"),
                    ("tb1", "t_b1"), ("ab0", "a_b0"), ("ab1", "a_b1")):
        a = f32a(inputs[src])
        for m in range(KT):
            putb((nm, m), a[m * 128 : (m + 1) * 128])
    finb = np.zeros(128, np.float32)
    for i in range(N):
        finb[i * 10 : i * 10 + 8] = f32a(inputs["a_b2"])[0]
        finb[i * 10 + 8] = f32a(inputs["c_b2"])[0]
        finb[i * 10 + 9] = f32a(inputs["t_b2"])[0]
    putb(("finb",), finb)

    ow0v = np.zeros((IN, N * H), bf)
    oW0 = f32a(inputs["o_W0"])
    for n in range(N):
        ow0v[:, n * H : (n + 1) * H] = oW0[n].astype(bf)

    xT = np.ascontiguousarray(f32a(inputs["x"]).T)  # (24, 4096)
    common = {"wb": wbv, "ow0": ow0v, "bias": biasv}
    in_maps = []
    for c in range(NCORES):
        m = dict(common)
        m["xT"] = np.ascontiguousarray(xT[:, c * BC : (c + 1) * BC]).astype(bf)
        in_maps.append(m)
    return in_maps


def run(inputs, trace=False, **kw):
    if "nc" not in _CACHE:
        _CACHE["nc"] = _build()
    nc = _CACHE["nc"]
    in_maps = _prep_inputs(inputs)
    res = run_bass_kernel_spmd(nc, in_maps, list(range(NCORES)), trace=trace, **kw)
    out = np.concatenate([res.results[c]["outT"].T for c in range(NCORES)], axis=0)
    return out.astype(np.float32), res


def kernel(**inputs) -> np.ndarray:
    out, _ = run(inputs, trace=False)
    return out


# revision 10
# speedup vs baseline: 1.5285x; 1.0146x over previous
"""BlockStackingSGN kernel for 8 Trainium2 NeuronCores.

Strategy: data-parallel over batch B=4096 -> 512 rows per core; all MLP
weights replicated. On-chip layout keeps activations transposed
([hidden -> partitions, batch -> free]) so every matmul streams the batch
through the PE with the weight stationary (bf16 operands, fp32 PSUM
accumulation). The three 256->1 output heads (clear / ontable / AonB) are
folded into one PSUM accumulation bank: each head's weight column is
embedded at output-row position r of a [128,128] stationary operand, so
all 80 output rows accumulate into a single [128, 512] bank and one
batched Sigmoid finishes the kernel. Elementwise work is spread across
the Scalar, Vector, and GpSimd engines to keep them all under the PE's
span; all 128-partition weights travel in one packed SBUF tile DMA'd in
parallel chunks.
"""

import sys

import numpy as np

sys.path.insert(0, "/opt/trn_rl_repo")

import concourse.bacc as bacc
import concourse.mybir as mybir
import concourse.tile as tile
from concourse.bass_utils import run_bass_kernel_spmd

dt = mybir.dt
AF = mybir.ActivationFunctionType
ALU = mybir.AluOpType

N = 8          # blocks
H = 256        # hidden
B = 4096       # batch
IN = 3 * N     # 24
NCORES = 8
BC = B // NCORES   # 512 batch rows per core
KT = H // 128      # k-tiles per 256-wide contraction
R = N * (N + 2)    # 80 output rows per batch element

F32 = dt.float32
BF16 = dt.bfloat16
W = BC

_CACHE = {}


def _wb_layout():
    """Column layout of the packed [128, ncols] bf16 weight tile.
    Order doubles as DMA arrival order: block-0 weights first."""
    keys = []
    for n in range(N):
        for k in range(KT):
            keys.append(("oW1", n, k))
            keys.append(("oW2", n, k))
    for nm in ("cW0", "cW1", "tW0", "tW1"):
        for k in range(KT):
            keys.append((nm, k))
    for nm in ("aW0l", "aW0r", "aW1"):
        for k in range(KT):
            keys.append((nm, k))
    for t_ in range(3):
        for k in range(KT):
            for p in range(2):
                keys.append(("w2e", t_, k, p))
    off = {}
    col = 0
    for key in keys:
        off[key] = col
        col += H
    return off, col


WB_OFF, WB_COLS = _wb_layout()

# bias tile column layout ([128, nb] fp32)
def _bias_layout():
    keys = []
    for n in range(N):
        for nm in ("ob0", "ob1", "ob2"):
            for m in range(KT):
                keys.append((nm, n, m))
    for nm in ("cb0", "cb1", "tb0", "tb1", "ab0", "ab1"):
        for m in range(KT):
            keys.append((nm, m))
    keys.append(("finb",))
    return {k: i for i, k in enumerate(keys)}, len(keys)


BIAS_OFF, BIAS_COLS = _bias_layout()

N_DMA_CHUNKS = 8


def _build():
    nc = bacc.Bacc("TRN2", target_bir_lowering=False, debug=False, num_devices=NCORES)

    d_xT = nc.dram_tensor("xT", [IN, BC], BF16, kind="ExternalInput")
    d_wb = nc.dram_tensor("wb", [128, WB_COLS], BF16, kind="ExternalInput")
    d_ow0 = nc.dram_tensor("ow0", [IN, N * H], BF16, kind="ExternalInput")
    d_bias = nc.dram_tensor("bias", [128, BIAS_COLS], F32, kind="ExternalInput")
    d_out = nc.dram_tensor("outT", [R, BC], F32, kind="ExternalOutput")

    with tile.TileContext(nc) as tc:
        with (
            tc.tile_pool(name="w", bufs=1) as wp,
            tc.tile_pool(name="act", bufs=1) as ap,
            tc.tile_pool(name="wk", bufs=6) as wk,
            tc.tile_pool(name="ps", bufs=7, space="PSUM") as ps,
            tc.tile_pool(name="finp", bufs=1, space="PSUM") as fp,
        ):
            xT = wp.tile([IN, BC], BF16, tag="xT")
            nc.sync.dma_start(xT[:], d_xT[:])
            ow0 = wp.tile([IN, N * H], BF16, tag="ow0")
            nc.gpsimd.dma_start(ow0[:], d_ow0[:])
            bias = wp.tile([128, BIAS_COLS], F32, tag="bias")
            nc.sync.dma_start(bias[:], d_bias[:])

            wb = wp.tile([128, WB_COLS], BF16, tag="wb")
            chunk = (WB_COLS + N_DMA_CHUNKS - 1) // N_DMA_CHUNKS
            for c in range(N_DMA_CHUNKS):
                eng = nc.gpsimd if c % 2 == 0 else nc.sync
                sl = slice(c * chunk, min((c + 1) * chunk, WB_COLS))
                eng.dma_start(wb[:, sl], d_wb[:, sl])

            def wslice(*key):
                o = WB_OFF[key]
                return lambda m: wb[:, o + m * 128 : o + (m + 1) * 128]

            def bcol(*key):
                i = BIAS_OFF[key]
                return bias[:, i : i + 1]

            fin = fp.tile([128, BC], F32, tag="fin")
            n_fin = 2 * KT * N + KT * N * N
            fin_ct = [0]

            def fin_mm(t_, k, r, rhs):
                first = fin_ct[0] == 0
                fin_ct[0] += 1
                last = fin_ct[0] == n_fin
                p = r % 2
                o = WB_OFF[("w2e", t_, k, p)]
                lhsT = wb[:, o + 128 + p - r : o + 256 + p - r]
                nc.tensor.matmul(fin[:], lhsT, rhs, start=first, stop=last)

            # weighted round-robin of PSUM evacuations; ratio set per phase
            evac_ct = [0]
            ev_pat = ["AD"]

            def evac(out_ap, psum_ap, bias_ap, relu):
                e = ev_pat[0][evac_ct[0] % len(ev_pat[0])]
                evac_ct[0] += 1
                if e == "A":
                    func = AF.Relu if relu else (AF.Identity if bias_ap is not None else AF.Copy)
                    if bias_ap is not None:
                        nc.scalar.activation(out_ap, psum_ap, func, bias=bias_ap)
                    else:
                        nc.scalar.activation(out_ap, psum_ap, func)
                else:
                    if relu:
                        b = bias_ap if bias_ap is not None else 0.0
                        nc.vector.tensor_scalar(out_ap, psum_ap, b, 0.0, ALU.add, ALU.max)
                    elif bias_ap is not None:
                        nc.vector.tensor_scalar(out_ap, psum_ap, bias_ap, None, ALU.add)
                    else:
                        nc.vector.tensor_copy(out_ap, psum_ap)

            def layer(wgt, bias_m, in_tile, in_parts, relu, out_pool, out_tag):
                """One 256-out layer -> [128, KT*W] tile.
                wgt(k) -> fn m -> lhsT AP; in_parts: list of rhs APs (k-tiles)."""
                out = out_pool.tile([128, KT * W], BF16, tag=out_tag)
                if in_parts is None:
                    in_parts = [in_tile[:, k * W : (k + 1) * W] for k in range(KT)]
                for m in range(KT):
                    pst = ps.tile([128, BC], F32, tag="ps")
                    for ki, rhs in enumerate(in_parts):
                        nc.tensor.matmul(pst[:], wgt(ki)(m), rhs,
                                         start=(ki == 0), stop=(ki == len(in_parts) - 1))
                    evac(out[:, m * W : (m + 1) * W], pst[:],
                         bias_m(m) if bias_m else None, relu)
                return out

            # ---- object encoders -> enc[n] [128, 2W] (persistent) ----
            enc = []
            for n in range(N):
                ow0_l = lambda n=n: (lambda ki: (lambda m: ow0[:, n * H + m * 128 : n * H + (m + 1) * 128]))
                h0 = layer(ow0_l(), lambda m, n=n: bcol("ob0", n, m), None, [xT[:]], True, wk, "h")
                h1 = layer(lambda ki, n=n: wslice("oW1", n, ki), lambda m, n=n: bcol("ob1", n, m),
                           h0, None, True, wk, "h")
                e = layer(lambda ki, n=n: wslice("oW2", n, ki), lambda m, n=n: bcol("ob2", n, m),
                          h1, None, False, ap, f"enc_{n}")
                enc.append(e)

            # ---- AonB pair-input halves (bias a_b0 folded into left) ----
            al, ar = [], []
            for n in range(N):
                al.append(layer(lambda ki: wslice("aW0l", ki), lambda m: bcol("ab0", m),
                                enc[n], None, False, ap, f"al_{n}"))
                ar.append(layer(lambda ki: wslice("aW0r", ki), None,
                                enc[n], None, False, ap, f"ar_{n}"))

            # ---- clear / ontable predicate thunks (interleaved into pairs) ----
            def pred_thunk(n, w0nm, w1nm, b0nm, b1nm, w2idx, r):
                def go():
                    y0 = layer(lambda ki: wslice(w0nm, ki),
                               lambda m: bcol(b0nm, m), enc[n], None, True, wk, "h")
                    y1 = layer(lambda ki: wslice(w1nm, ki),
                               lambda m: bcol(b1nm, m), y0, None, True, wk, "h")
                    for k in range(KT):
                        fin_mm(w2idx, k, r, y1[:, k * W : (k + 1) * W])
                return go

            preds = []
            for n in range(N):
                preds.append(pred_thunk(n, "cW0", "cW1", "cb0", "cb1", 0, n * 10 + 8))
                preds.append(pred_thunk(n, "tW0", "tW1", "tb0", "tb1", 1, n * 10 + 9))

            # ---- all (i, j) pairs, ordered round-robin across the three
            # 32-row PSUM partition groups ----
            zero = wp.tile([128, KT * W], BF16, tag="zero")
            nc.gpsimd.memset(zero[:], 0.0)
            ev_pat[0] = "AAAAAD"  # pair phase: DVE busy with add/relu chains
            buckets = [[], [], []]
            for i in range(N):
                for j in range(N):
                    buckets[(i * 10 + j) // 32].append((i, j))
            order = []
            bi = 0
            while any(buckets):
                if buckets[bi % 3]:
                    order.append(buckets[bi % 3].pop(0))
                bi += 1
            for pi, (i, j) in enumerate(order):
                if pi % 4 == 0 and preds:
                    preds.pop(0)()
                r = i * 10 + j
                phs = wk.tile([128, KT * W], BF16, tag="phs")
                nc.vector.tensor_tensor(phs[:], al[i][:], ar[j][:], ALU.add)
                ph = wk.tile([128, KT * W], BF16, tag="ph")
                nc.vector.tensor_tensor(ph[:], phs[:], zero[:], ALU.max)
                y = layer(lambda ki: wslice("aW1", ki), lambda m: bcol("ab1", m),
                          ph, None, True, wk, "y")
                for k in range(KT):
                    fin_mm(2, k, r, y[:, k * W : (k + 1) * W])
            for t in preds:
                t()

            assert fin_ct[0] == n_fin

            # ---- batched sigmoid over all 80 head rows + store ----
            outT = wk.tile([128, BC], F32, tag="outT")
            nc.scalar.activation(outT[:], fin[:], AF.Sigmoid, bias=bcol("finb"))
            nc.sync.dma_start(d_out[:], outT[:R, :])

    nc.compile()
    return nc


def _prep_inputs(inputs):
    import ml_dtypes

    bf = ml_dtypes.bfloat16
    f32a = lambda a: np.asarray(a, dtype=np.float32)

    wbv = np.zeros((128, WB_COLS), bf)

    def put(key, arr):  # arr: [128, 256] fp32
        o = WB_OFF[key]
        wbv[:, o : o + H] = arr.astype(bf)

    oW1 = f32a(inputs["o_W1"])
    oW2 = f32a(inputs["o_W2"])
    for n in range(N):
        for k in range(KT):
            put(("oW1", n, k), oW1[n, k * 128 : (k + 1) * 128])
            put(("oW2", n, k), oW2[n, k * 128 : (k + 1) * 128])
    for nm, src in (("cW0", "c_W0"), ("cW1", "c_W1"), ("tW0", "t_W0"), ("tW1", "t_W1")):
        a = f32a(inputs[src])
        for k in range(KT):
            put((nm, k), a[k * 128 : (k + 1) * 128])
    aW0 = f32a(inputs["a_W0"])
    for k in range(KT):
        put(("aW0l", k), aW0[k * 128 : (k + 1) * 128])
        put(("aW0r", k), aW0[H + k * 128 : H + (k + 1) * 128])
    aW1 = f32a(inputs["a_W1"])
    for k in range(KT):
        put(("aW1", k), aW1[k * 128 : (k + 1) * 128])
    for t_, src2 in enumerate(("c_W2", "t_W2", "a_W2")):
        w2 = f32a(inputs[src2])[:, 0].astype(bf)
        for k in range(KT):
            for p in range(2):
                o = WB_OFF[("w2e", t_, k, p)]
                wbv[:, o + 128 + p] = w2[k * 128 : (k + 1) * 128]

    biasv = np.zeros((128, BIAS_COLS), np.float32)

    def putb(key, vec128):
        biasv[:, BIAS_OFF[key]] = vec128

    for n in range(N):
        for nm, src in (("ob0", "o_b0"), ("ob1", "o_b1"), ("ob2", "o_b2")):
            a = f32a(inputs[src])[n]
            for m in range(KT):
                putb((nm, n, m), a[m * 128 : (m + 1) * 128])
    for nm, src in (("cb0", "c_b0"), ("cb1", "c_b1"), ("tb0", "t_b0"),
                    ("tb1", "t_b1"), ("ab0", "a_b0"), ("ab1", "a_b1")):
        a = f32a(inputs[src])
        for m in range(KT):
            putb((nm, m), a[m * 128 : (m + 1) * 128])
    finb = np.zeros(128, np.float32)
    for i in range(N):
        finb[i * 10 : i * 10 + 8] = f32a(inputs["a_b2"])[0]
        finb[i * 10 + 8] = f32a(inputs["c_b2"])[0]
        finb[i * 10 + 9] = f32a(inputs["t_b2"])[0]
    putb(("finb",), finb)

    ow0v = np.zeros((IN, N * H), bf)
    oW0 = f32a(inputs["o_W0"])
    for n in range(N):
        ow0v[:, n * H : (n + 1) * H] = oW0[n].astype(bf)

    xT = np.ascontiguousarray(f32a(inputs["x"]).T)  # (24, 4096)
    common = {"wb": wbv, "ow0": ow0v, "bias": biasv}
    in_maps = []
    for c in range(NCORES):
        m = dict(common)
        m["xT"] = np.ascontiguousarray(xT[:, c * BC : (c + 1) * BC]).astype(bf)
        in_maps.append(m)
    return in_maps


def run(inputs, trace=False, **kw):
    if "nc" not in _CACHE:
        _CACHE["nc"] = _build()
    nc = _CACHE["nc"]
    in_maps = _prep_inputs(inputs)
    res = run_bass_kernel_spmd(nc, in_maps, list(range(NCORES)), trace=trace, **kw)
    out = np.concatenate([res.results[c]["outT"].T for c in range(NCORES)], axis=0)
    return out.astype(np.float32), res


def kernel(**inputs) -> np.ndarray:
    out, _ = run(inputs, trace=False)
    return out
